# revision 14
# baseline (speedup 1.0000x reference)
"""Trainium2 Bass kernel for nn_BayerFeatureExtractor.

Computes 52 feature channels from a [2,1,768,768] bayer image, data-parallel
over 8 NeuronCores (each core: one batch image x 192 rows, 2 row-blocks).

Strategy:
  - Host reflect-pads each batch image by (3 rows, 6 cols); each core gets a
    [198, 780] fp32r strip (rows on SBUF partitions).
  - All convolutions run on the TensorEngine as banded matmuls (fp32r, full
    rate): contraction over input rows with a banded weight matrix encoding
    the kernel's row profile; one PSUM-accumulated pass per nonzero kernel
    column, with the moving operand shifted along the free (column) dim.
  - Intermediates that feed 3x3 box filters are computed on an "E" grid
    extended by 1 row/col so the second conv stage needs no partition-offset
    reads (compute engines require partition start 0); reflect behavior of
    intermediates at image borders is exact because every relevant kernel
    profile is symmetric (or enters squared).
  - Pointwise math spread across Vector (DVE), Scalar (ACT), GPSIMD engines.
  - Morphological gradient via 3 row-shifted DMA copies + max/min chains.
"""
import sys
import math

sys.path.insert(0, '/opt/trn_rl_repo')

import numpy as np

EPS = 1e-6

H = 768
W = 768
B = 2
NCORES = 8
CORES_PER_BATCH = 4
RPC = H // CORES_PER_BATCH          # 192 output rows per core
NBLK = 2
BR = RPC // NBLK                    # 96 output rows per block
PH = 3                              # host row padding
PW = 6                              # host col padding
SR = RPC + 2 * PH                   # 198 strip rows
SW = W + 2 * PW                     # 780 strip cols
KIN = BR + 2 * PH                   # 102 contraction rows per block
ME, MO = 98, 96                     # E-grid / O-grid matmul M
NE, NO = 386, 384                   # matmul half widths
EW = 772                            # E tile width (covers out cols -2..769)


# ---------------------------------------------------------------- kernels ---
def _npk(a, s=1.0):
    return np.asarray(a, dtype=np.float32) * np.float32(s)


def _gabor(theta, sigma=1.1, lambd=3.0, gamma=0.65):
    c = np.arange(-2, 3, dtype=np.float32)
    yy, xx = np.meshgrid(c, c, indexing='ij')
    xt = xx * math.cos(theta) + yy * math.sin(theta)
    yt = -xx * math.sin(theta) + yy * math.cos(theta)
    k = np.exp(-(xt ** 2 + gamma ** 2 * yt ** 2) / (2.0 * sigma ** 2)) * np.cos(
        2.0 * math.pi * xt / lambd)
    k = k - k.mean()
    return (k / max(np.abs(k).sum(), 1e-6)).astype(np.float32)


def _dct_like(u=2, v=2, size=5):
    c = np.arange(size, dtype=np.float32)
    yy, xx = np.meshgrid(c, c, indexing='ij')
    k = np.cos(math.pi * (2 * xx + 1) * u / (2 * size)) * np.cos(
        math.pi * (2 * yy + 1) * v / (2 * size))
    k = k - k.mean()
    return (k / max(np.abs(k).sum(), 1e-6)).astype(np.float32)


K_LAP = _npk([[0, 1, 0], [1, -4, 1], [0, 1, 0]])
K_HXX = _npk([[1, -2, 1]])                     # row kernel
K_HYY = _npk([[1], [-2], [1]])                 # col kernel
K_HXY = _npk([[1, 0, -1], [0, 0, 0], [-1, 0, 1]], 0.25)
K_GX = _npk([[-1, 0, 1], [-2, 0, 2], [-1, 0, 1]], 0.125)
K_GY = _npk([[-1, -2, -1], [0, 0, 0], [1, 2, 1]], 0.125)
K_GDM = _npk([[-2, -1, 0], [-1, 0, 1], [0, 1, 2]], 0.125)
K_GDA = _npk([[0, 1, 2], [-1, 0, 1], [-2, -1, 0]], 0.125)
K_CHK = _npk([[1, -1, 1], [-1, 1, -1], [1, -1, 1]], 1.0 / 9.0)

HGH_V = np.array([-0.25, 0.5, 0.5, 0.5, -0.25], np.float32)
K_HSH = _npk([[-0.5, 0.0, 1.0, 0.0, -0.5]])
K_HSV = _npk([[-0.5], [0.0], [1.0], [0.0], [-0.5]])
K_HGH = HGH_V.reshape(1, 5)
K_HGV = HGH_V.reshape(5, 1)
K_MHC = _npk([[0, 0, -1, 0, 0], [0, 0, 2, 0, 0], [-1, 2, 4, 2, -1],
              [0, 0, 2, 0, 0], [0, 0, -1, 0, 0]], 0.125)
K_STX = _npk([[0.25, -1.0, 1.5, -1.0, 0.25]])
K_STY = K_STX.reshape(5, 1).copy()
K_G45 = _gabor(math.pi / 4.0)
K_G135 = _gabor(3.0 * math.pi / 4.0)
K_DCT = _dct_like()
K_SMOOTH5 = (_npk([[1, 2, 3, 2, 1], [2, 4, 6, 4, 2], [3, 6, 9, 6, 3],
                   [2, 4, 6, 4, 2], [1, 2, 3, 2, 1]]) / np.float32(81.0))
K_RESH = (np.eye(1, 5, 2, dtype=np.float32) - K_HGH)      # delta - hgh (1x5)
K_RESV = (np.eye(5, 1, -2, dtype=np.float32) - K_HGV)     # delta - hgv (5x1)
K_BOX3 = np.full((3, 3), 1.0 / 9.0, np.float32)
K_AVGH5 = np.full((1, 5), 0.2, np.float32)
K_AVGV5 = np.full((5, 1), 0.2, np.float32)


def _mask_pattern(name):
    # value at (row parity, col parity), gbrg pattern
    m = np.zeros((2, 2), np.float32)
    if name == 'r':
        m[1, 0] = 1.0
    elif name == 'b':
        m[0, 1] = 1.0
    elif name == 'gr':
        m[1, 1] = 1.0
    elif name == 'gb':
        m[0, 0] = 1.0
    elif name == 'g':
        m[0, 0] = 1.0; m[1, 1] = 1.0
    elif name == 'row':
        m[1, :] = 1.0
    elif name == 'col':
        m[:, 1] = 1.0
    return m


def _den_pattern(name):
    # conv(mask, SMOOTH5) is exactly 2x2-periodic (reflect == parity ext.)
    pat = _mask_pattern(name)
    g = np.zeros((16, 16), np.float32)
    for r in range(16):
        for c in range(16):
            g[r, c] = pat[r % 2, c % 2]
    out = np.zeros((2, 2), np.float32)
    for r in (6, 7):
        for c in (6, 7):
            acc = np.float32(0.0)
            for dy in range(5):
                for dx in range(5):
                    acc += K_SMOOTH5[dy, dx] * g[r + dy - 2, c + dx - 2]
            out[r % 2, c % 2] = acc
    return np.maximum(out, EPS)


def _tile_pattern(pat, rows, cols, row_shift=0, col_shift=0):
    out = np.zeros((rows, cols), np.float32)
    for rp in range(2):
        for cp in range(2):
            out[rp::2, cp::2] = pat[(rp + row_shift) % 2, (cp + col_shift) % 2]
    return out


# ------------------------------------------------------------ band builder ---
class Bands:
    """Dedup banded lhsT matrices per grid ('O' out rows, 'E' extended, 'B' box)."""

    def __init__(self):
        self.items = {'O': [], 'E': [], 'B': []}
        self.index = {}

    def get(self, grid, prof):
        key = (grid, tuple(np.round(np.asarray(prof, np.float64), 10)))
        if key in self.index:
            return self.index[key]
        prof = np.asarray(prof, np.float32)
        kh = len(prof)
        off = kh // 2
        if grid == 'O':
            m = np.zeros((KIN, MO), np.float32)
            for mm in range(MO):
                base = mm + PH - off
                for t in range(kh):
                    m[base + t, mm] = prof[t]
        elif grid == 'E':
            m = np.zeros((KIN, ME), np.float32)
            for ii in range(ME):
                base = ii + PH - 1 - off
                for t in range(kh):
                    m[base + t, ii] = prof[t]
        else:  # 'B': 3-row box applied to E tiles
            m = np.zeros((ME, MO), np.float32)
            for mm in range(MO):
                for t in range(kh):
                    m[mm + t, mm] = prof[t]
        idx = len(self.items[grid])
        self.items[grid].append(m)
        self.index[key] = idx
        return idx

    def passes(self, grid, K):
        K = np.atleast_2d(np.asarray(K, np.float32))
        kw = K.shape[1]
        out = []
        for dxi in range(kw):
            col = K[:, dxi]
            if np.any(col != 0.0):
                out.append((dxi - kw // 2, self.get(grid, col)))
        return out


_BANDS = Bands()

P_E = {
    'gx': _BANDS.passes('E', K_GX),
    'gy': _BANDS.passes('E', K_GY),
    'chk': _BANDS.passes('E', K_CHK),
    'stx': _BANDS.passes('E', K_STX),
    'sty': _BANDS.passes('E', K_STY),
    'resh': _BANDS.passes('E', K_RESH),
    'resv': _BANDS.passes('E', K_RESV),
}
P_O = {
    'lap': _BANDS.passes('O', K_LAP),
    'hxx': _BANDS.passes('O', K_HXX),
    'hyy': _BANDS.passes('O', K_HYY),
    'hxy': _BANDS.passes('O', K_HXY),
    'gdm': _BANDS.passes('O', K_GDM),
    'gda': _BANDS.passes('O', K_GDA),
    'hsh': _BANDS.passes('O', K_HSH),
    'hsv': _BANDS.passes('O', K_HSV),
    'hgh': _BANDS.passes('O', K_HGH),
    'hgv': _BANDS.passes('O', K_HGV),
    'mhcf': _BANDS.passes('O', K_MHC),
    'g45': _BANDS.passes('O', K_G45),
    'g135': _BANDS.passes('O', K_G135),
    'dct': _BANDS.passes('O', K_DCT),
    'sm5': _BANDS.passes('O', K_SMOOTH5),
    'avgh5': _BANDS.passes('O', K_AVGH5),
    'avgv5': _BANDS.passes('O', K_AVGV5),
    'box3': _BANDS.passes('O', K_BOX3),
}
P_B = _BANDS.passes('B', K_BOX3)


BANDS_E = np.stack(_BANDS.items['E'])          # [nE, 102, 98]
def _trunc10(a):
    b = np.asarray(a, np.float32).copy()
    v = b.view(np.uint32)
    v &= np.uint32(0xFFFFE000)
    return b


K_AVGH5_H = _trunc10(K_AVGH5)
K_AVGH5_L = K_AVGH5 - K_AVGH5_H
K_AVGV5_H = _trunc10(K_AVGV5)
K_AVGV5_L = K_AVGV5 - K_AVGV5_H
P_O.update({
    'avgh5_h': _BANDS.passes('O', K_AVGH5_H),
    'avgh5_l': _BANDS.passes('O', K_AVGH5_L),
    'avgv5_h': _BANDS.passes('O', K_AVGV5_H),
    'avgv5_l': _BANDS.passes('O', K_AVGV5_L),
})

# --- polyphase smooth5 bands: row-parity mask and 1/den folded into bands.
# For mask X (row parity rX, col parity cX) and den channel D, out-col phase
# p: passes use only taps dx with (p+dx)%2 == cX; band rows of wrong parity
# are zeroed; band cols scaled by 1/den_D(out-row parity, p).
_SM5_META = {'r': (1, 0), 'b': (0, 1), 'gr': (1, 1), 'gb': (0, 0)}


def _sm5_passes(X, D):
    rX, cX = _SM5_META[X]
    rden = 1.0 / _den_pattern(D)
    out = []
    for p in range(2):
        for dxi in range(5):
            if (p + dxi) % 2 != cX:
                continue
            m = np.zeros((KIN, MO), np.float32)
            for mm in range(MO):
                for t in range(5):
                    k = mm + 1 + t
                    if (k + 1) % 2 == rX:
                        m[k, mm] = K_SMOOTH5[t, dxi] * rden[mm % 2, p]
            key = ('SM5', X, D, p, dxi)
            if key not in _BANDS.index:
                _BANDS.index[key] = len(_BANDS.items['O'])
                _BANDS.items['O'].append(m)
            # rhs strided-view offset (in cX-parity column units)
            j0 = (PW + p + (dxi - 2) - cX) // 2
            out.append((p, _BANDS.index[key], j0))
    return out


P_SM5 = {
    'rf': (_sm5_passes('r', 'r'), 0),
    'bf': (_sm5_passes('b', 'b'), 1),
    'grf': (_sm5_passes('gr', 'gr'), 1),
    'gbf': (_sm5_passes('gb', 'gb'), 0),
}
P_SM5_GF = (_sm5_passes('gr', 'g'), _sm5_passes('gb', 'g'))  # accumulate both

BANDS_O = np.stack(_BANDS.items['O'])          # [nO, 102, 96]
_BB_STD = _BANDS.items['B'][0]
BANDS_B = np.stack([_BB_STD, _BB_STD, _BB_STD])   # [3, 98, 96]: std, top-slot, bot-slot


def _bb_variant(kind):
    m = _BB_STD.copy()
    if kind == 'top':
        m[0, 0] = -m[0, 0]
    else:
        m[ME - 1, MO - 1] = -m[ME - 1, MO - 1]
    return m

CH = {n: i for i, n in enumerate([
    'r', 'g', 'b', 'gr', 'gb', 'rowm', 'colm',
    'lap', 'hxx', 'hyy', 'hxy', 'mgrad', 'gx', 'gy', 'gdm', 'gda', 'gmag',
    'coherence', 'anisotropy', 'hsh', 'hsv', 'hgh', 'hgv', 'ha_dis',
    'res_h', 'res_v', 'res_eh', 'res_ev', 'dgd', 'dsd', 'lvh', 'lvv', 'lvd',
    'dconf', 'rg', 'bg', 'gpd', 'mhc', 'mhc_ha', 'rres', 'bres',
    'stx', 'sty', 'chk', 'g45', 'g135', 'dctp', 'chk_e', 'str_e', 'lmean',
    'lvar', 'gen'])}


# ------------------------------------------------------------- bass program ---
_PROGRAM = {}


def _build_program(loop=1, timing=False):
    import concourse.bacc as bacc
    import concourse.mybir as mybir
    from concourse.tile import TileContext

    f32 = mybir.dt.float32
    f32r = mybir.dt.float32r
    u8 = mybir.dt.uint8
    A = mybir.AluOpType
    AF = mybir.ActivationFunctionType

    nc = bacc.Bacc("TRN2")

    if timing:
        def declare(name, shape, dtype, isOutput):
            return nc.dram_tensor(name, shape, dtype).ap()
        tin = nc.declare_dram_parameter("tin", [1, 4], mybir.dt.float32, isOutput=False)
        tout = nc.declare_dram_parameter("tout", [1, 4], mybir.dt.float32, isOutput=True)
    else:
        def declare(name, shape, dtype, isOutput):
            return nc.declare_dram_parameter(name, shape, dtype, isOutput=isOutput)

    def register_const(value):
        t = nc.alloc_sbuf_tensor(f"constf32-{value}", [128, 1], f32)
        nc.gpsimd.memset(t.ap(), value)
        nc.const_aps.aps[(f32, value)] = t.ap()

    register_const(EPS)
    nc.all_engine_barrier()

    nO, nE, nB = BANDS_O.shape[0], BANDS_E.shape[0], BANDS_B.shape[0]
    xs_ext = declare("xs", [SR, SW], f32r, isOutput=False)
    bo_ext = declare("bandsO", [KIN, nO * MO], f32r, isOutput=False)
    be_ext = declare("bandsE", [KIN, nE * ME], f32r, isOutput=False)
    bb_ext = declare("bandsB", [ME, nB * MO], f32r, isOutput=False)
    rbmask_ext = declare("rbmask", [MO, 2 * W], f32, isOutput=False)
    gmask_ext = declare("gmask", [MO, W], f32, isOutput=False)
    masks7_ext = declare("masks7", [7, MO, W], f32, isOutput=False)
    out_ext = declare("out", [52, RPC, W], f32, isOutput=True)

    with TileContext(nc) as tc:
        with (
            tc.tile_pool(name="const", bufs=1) as cpool,
            tc.tile_pool(name="work", bufs=1) as wpool,
            tc.tile_pool(name="ebuf", bufs=8) as epool,
            tc.tile_pool(name="obuf", bufs=15) as opool,
            tc.tile_pool(name="psE", bufs=2, space="PSUM") as ppe,
            tc.tile_pool(name="psO", bufs=2, space="PSUM") as ppo,
        ):
            # ---- constants -> SBUF (once) ----
            bo_t = cpool.tile([KIN, nO * MO], f32r)
            nc.sync.dma_start(out=bo_t[:], in_=bo_ext[:])
            be_t = cpool.tile([KIN, nE * ME], f32r)
            nc.sync.dma_start(out=be_t[:], in_=be_ext[:])
            bb_t = cpool.tile([ME, nB * MO], f32r)
            nc.sync.dma_start(out=bb_t[:], in_=bb_ext[:])
            rbmask_t = cpool.tile([MO, 2 * W], f32)
            nc.sync.dma_start(out=rbmask_t[:], in_=rbmask_ext[:])
            gmask_t = cpool.tile([MO, W], f32)
            nc.sync.dma_start(out=gmask_t[:], in_=gmask_ext[:])

            def bandO(i):
                return bo_t[:, i * MO:(i + 1) * MO]

            def bandE(i):
                return be_t[:, i * ME:(i + 1) * ME]

            def bandB(i):
                return bb_t[:, i * MO:(i + 1) * MO]

            def h3(ap):
                # [P, 2N] (possibly offset slice) -> [P, 2, N]
                return ap.rearrange("p (b n) -> p b n", b=2)

            if timing:
                nc.sync.dma_start(out=tout[:], in_=tin[:])
            # mask output channels: DRAM -> DRAM
            for ch_i, name in enumerate(['r', 'g', 'b', 'gr', 'gb', 'rowm', 'colm']):
                for blk in range(NBLK):
                    nc.sync.dma_start(out=out_ext[CH[name], blk * BR:(blk + 1) * BR, :],
                                      in_=masks7_ext[ch_i])

            import contextlib
            loop_cm = tc.For_i(0, loop, 1) if loop > 1 else contextlib.nullcontext()
            with loop_cm:
              for blk in range(NBLK):
                  r0 = blk * BR

                  # ---- inputs ----
                  strip = wpool.tile([KIN, SW], f32r, tag="strip", bufs=2)
                  nc.scalar.dma_start(out=strip[:], in_=xs_ext[r0:r0 + KIN, :])
                  stripf = strip[:].bitcast(f32)
                  T0 = wpool.tile([MO, SW], f32, tag="T0", bufs=2)
                  T1 = wpool.tile([MO, SW], f32, tag="T1", bufs=2)
                  T2 = wpool.tile([MO, SW], f32, tag="T2", bufs=2)
                  nc.scalar.dma_start(out=T0[:], in_=xs_ext[r0 + 2:r0 + 2 + MO, :].bitcast(f32))
                  nc.scalar.dma_start(out=T1[:], in_=xs_ext[r0 + 3:r0 + 3 + MO, :].bitcast(f32))
                  nc.scalar.dma_start(out=T2[:], in_=xs_ext[r0 + 4:r0 + 4 + MO, :].bitcast(f32))

                  bayerO = T1[:, PW:PW + W]

                  # ---- derived matmul inputs ----
                  # strip32 load dropped: strip f32r bytes ARE the f32 values
                  bsq32 = wpool.tile([KIN, SW], f32, tag="bsq32")
                  nc.scalar.activation(bsq32[:], stripf, AF.Square)
                  # hi/lo fp32r split of bayer and bayer^2 for full-precision
                  # variance convs at fp32r matmul rate
                  xh_t = wpool.tile([KIN, SW], f32r, tag="xh")
                  nc.scalar.copy(out=xh_t[:], in_=stripf)
                  xl_t = wpool.tile([KIN, SW], f32r, tag="xl")
                  nc.vector.tensor_sub(out=xl_t[:], in0=stripf, in1=xh_t[:].bitcast(f32))
                  bsqr = wpool.tile([KIN, SW], f32r, tag="bsqr")
                  nc.scalar.activation(bsqr[:], stripf, AF.Square)
                  bsql = wpool.tile([KIN, SW], f32r, tag="bsql")
                  nc.vector.tensor_sub(out=bsql[:], in0=bsq32[:], in1=bsqr[:].bitcast(f32))

                  # ---- conv helpers ----
                  def conv(passes, grid, rhs, band_fn, M, NH):
                      pool = ppe if grid == 'E' else ppo
                      ps = pool.tile([M, 1024], f32, tag="pe" if grid == 'E' else "po",
                                     name="ps")
                      shift = {'E': PW - 2, 'O': PW, 'B': 2}[grid]
                      for h in range(2):
                          for i, (dx, bi) in enumerate(passes):
                              nc.tensor.matmul(
                                  ps[:, h * 512:h * 512 + NH],
                                  band_fn(bi),
                                  rhs[:, shift + dx + h * NH: shift + dx + h * NH + NH],
                                  start=(i == 0), stop=(i == len(passes) - 1))
                      return ps

                  def convE(name, rhs=None):
                      return conv(P_E[name], 'E', (rhs if rhs is not None else strip)[:], bandE, ME, NE)

                  def convO(name, rhs=None):
                      return conv(P_O[name], 'O', (rhs if rhs is not None else strip)[:], bandO, MO, NO)

                  def convB(rhs_tile):
                      return conv(P_B, 'B', rhs_tile[:ME], bandB, MO, NO)

                  def conv_split(kh_name, kl_name, xh, xl):
                      ps = ppo.tile([MO, 1024], f32, tag="po", name="ps")
                      for h in range(2):
                          chain = ([(bi, xh, dx) for dx, bi in P_O[kh_name]]
                                   + [(bi, xl, dx) for dx, bi in P_O[kh_name]]
                                   + [(bi, xh, dx) for dx, bi in P_O[kl_name]])
                          for i, (bi, rhs_t, dx) in enumerate(chain):
                              nc.tensor.matmul(
                                  ps[:, h * 512:h * 512 + NO],
                                  bandO(bi),
                                  rhs_t[:, PW + dx + h * NO: PW + dx + h * NO + NO],
                                  start=(i == 0), stop=(i == len(chain) - 1))
                      return ps


                  def ps3(ps, NH):
                      return h3(ps[:, 0:1024])[:, :, 0:NH]

                  def e_named(tag, dtype=f32):
                      return wpool.tile([ME, EW], dtype, tag=tag, name=tag)

                  def e_roll(dtype=f32):
                      return epool.tile([ME, EW], dtype, tag="ebuf", name="eb")

                  def o_new():
                      return opool.tile([MO, W], f32, tag="obuf", name="ob")

                  def dma_out(name, ap):
                      nc.sync.dma_start(out=out_ext[CH[name], r0:r0 + BR, :], in_=ap)

                  def o_copy_out(name, ps, eng='v'):
                      t = o_new()
                      if eng == 'v':
                          nc.vector.tensor_copy(out=h3(t[:]), in_=ps3(ps, NO))
                      else:
                          nc.scalar.copy(out=h3(t[:]), in_=ps3(ps, NO))
                      dma_out(name, t[:])
                      return t

                  def e_win(t):
                      return t[1:97, 2:770]

                  # ---- E-grid stage 1 + products ----
                  gx_ps = convE('gx')
                  gy_ps = convE('gy')
                  gyS = e_named("gyS")
                  nc.vector.tensor_copy(out=h3(gyS[:]), in_=ps3(gy_ps, NE))
                  dma_out('gy', e_win(gyS))
                  gxx = e_named("gxx", f32r)
                  nc.scalar.activation(h3(gxx[:]), ps3(gx_ps, NE), AF.Square)
                  gyy = e_named("gyy", f32r)
                  nc.scalar.activation(gyy[:], gyS[:], AF.Square)
                  gxy = e_named("gxy", f32r)
                  nc.vector.tensor_mul(out=h3(gxy[:]), in0=ps3(gx_ps, NE), in1=h3(gyS[:]))
                  # virtual cols -1/768 of the gx*gy product have flipped sign
                  # relative to the reflect of the product; fix before the box.
                  nc.vector.tensor_scalar_mul(out=gxy[:, 1:2], in0=gxy[:, 1:2], scalar1=-1.0)
                  nc.vector.tensor_scalar_mul(out=gxy[:, 770:771], in0=gxy[:, 770:771], scalar1=-1.0)
                  absgx = e_roll()
                  nc.scalar.activation(h3(absgx[:]), ps3(gx_ps, NE), AF.Abs)
                  gxS = e_roll()
                  nc.scalar.copy(out=h3(gxS[:]), in_=ps3(gx_ps, NE))
                  dma_out('gx', e_win(gxS))
                  absgy = e_roll()
                  nc.scalar.activation(absgy[:], gyS[:], AF.Abs)
                  dgd = e_roll()
                  nc.gpsimd.tensor_sub(out=dgd[:], in0=absgx[:], in1=absgy[:])
                  dma_out('dgd', e_win(dgd))
                  g2 = e_roll()
                  nc.gpsimd.tensor_add(out=g2[:], in0=gxx[:].bitcast(f32), in1=gyy[:].bitcast(f32))
                  gmag = e_roll()
                  nc.scalar.activation(gmag[:], g2[:], AF.Sqrt, bias=EPS)
                  dma_out('gmag', e_win(gmag))

                  chk_ps = convE('chk')
                  chksq = e_named("chksq", f32r)
                  nc.scalar.activation(h3(chksq[:]), ps3(chk_ps, NE), AF.Square)
                  chkS = e_roll()
                  nc.scalar.copy(out=h3(chkS[:]), in_=ps3(chk_ps, NE))
                  dma_out('chk', e_win(chkS))

                  stx_ps = convE('stx')
                  stxsq = e_named("stxsq")
                  nc.scalar.activation(h3(stxsq[:]), ps3(stx_ps, NE), AF.Square)
                  stxS = e_roll()
                  nc.scalar.copy(out=h3(stxS[:]), in_=ps3(stx_ps, NE))
                  dma_out('stx', e_win(stxS))
                  sty_ps = convE('sty')
                  stysq = e_named("stysq")
                  nc.scalar.activation(h3(stysq[:]), ps3(sty_ps, NE), AF.Square)
                  styS = e_roll()
                  nc.vector.tensor_copy(out=h3(styS[:]), in_=ps3(sty_ps, NE))
                  dma_out('sty', e_win(styS))
                  s2 = e_named("s2", f32r)
                  nc.gpsimd.tensor_add(out=s2[:], in0=stxsq[:], in1=stysq[:])

                  resh_ps = convE('resh')
                  rhsq = e_named("rhsq", f32r)
                  nc.scalar.activation(h3(rhsq[:]), ps3(resh_ps, NE), AF.Square)
                  reshS = e_roll()
                  nc.vector.tensor_copy(out=h3(reshS[:]), in_=ps3(resh_ps, NE))
                  dma_out('res_h', e_win(reshS))
                  resv_ps = convE('resv')
                  rvsq = e_named("rvsq", f32r)
                  nc.scalar.activation(h3(rvsq[:]), ps3(resv_ps, NE), AF.Square)
                  resvS = e_roll()
                  nc.vector.tensor_copy(out=h3(resvS[:]), in_=ps3(resv_ps, NE))
                  dma_out('res_v', e_win(resvS))

                  # ---- mgrad (GPSIMD) ----
                  v1 = wpool.tile([MO, SW], f32, tag="mgtmp")
                  nc.vector.tensor_max(out=v1[:], in0=T0[:], in1=T2[:])
                  v3 = wpool.tile([MO, SW], f32, tag="v3")
                  nc.vector.tensor_max(out=v3[:], in0=v1[:], in1=T1[:])
                  n1 = wpool.tile([MO, SW], f32, tag="mgtmp")
                  nc.vector.tensor_tensor(out=n1[:], in0=T0[:], in1=T2[:], op=A.min)
                  n3 = wpool.tile([MO, SW], f32, tag="n3")
                  nc.vector.tensor_tensor(out=n3[:], in0=n1[:], in1=T1[:], op=A.min)
                  wm1 = wpool.tile([MO, W], f32, tag="mgw")
                  nc.vector.tensor_max(out=wm1[:], in0=v3[:, 5:5 + W], in1=v3[:, 6:6 + W])
                  wmx = wpool.tile([MO, W], f32, tag="wmx")
                  nc.vector.tensor_max(out=wmx[:], in0=wm1[:], in1=v3[:, 7:7 + W])
                  nm1 = wpool.tile([MO, W], f32, tag="mgw")
                  nc.vector.tensor_tensor(out=nm1[:], in0=n3[:, 5:5 + W], in1=n3[:, 6:6 + W], op=A.min)
                  nmn = wpool.tile([MO, W], f32, tag="nmn")
                  nc.vector.tensor_tensor(out=nmn[:], in0=nm1[:], in1=n3[:, 7:7 + W], op=A.min)
                  mgrad = o_new()
                  nc.gpsimd.tensor_sub(out=mgrad[:], in0=wmx[:], in1=nmn[:])
                  dma_out('mgrad', mgrad[:])

                  # ---- structure tensor boxes ----
                  jxx_ps = convB(gxx)
                  jyy_ps = convB(gyy)
                  jyyS = o_new()
                  nc.scalar.copy(out=h3(jyyS[:]), in_=ps3(jyy_ps, NO))
                  tr = o_new()
                  nc.vector.tensor_add(out=h3(tr[:]), in0=ps3(jxx_ps, NO), in1=h3(jyyS[:]))
                  dma_out('gen', tr[:])
                  dd = o_new()
                  nc.vector.tensor_sub(out=h3(dd[:]), in0=ps3(jxx_ps, NO), in1=h3(jyyS[:]))
                  pBj = [(dx, 1 + blk) for dx, _ in P_B]
                  jxy_ps = conv(pBj, 'B', gxy[:ME], bandB, MO, NO)
                  jxyS = o_new()
                  nc.scalar.copy(out=h3(jxyS[:]), in_=ps3(jxy_ps, NO))
                  d2 = o_new()
                  nc.scalar.activation(d2[:], dd[:], AF.Square)
                  jxy2 = o_new()
                  nc.scalar.activation(jxy2[:], jxyS[:], AF.Square)
                  ss = o_new()
                  nc.vector.scalar_tensor_tensor(out=ss[:], in0=jxy2[:], scalar=4.0, in1=d2[:],
                                                 op0=A.mult, op1=A.add)
                  lam = o_new()
                  nc.scalar.activation(lam[:], ss[:], AF.Sqrt, bias=EPS)
                  tre = o_new()
                  nc.vector.tensor_scalar_add(tre[:], tr[:], EPS)
                  rtr = o_new()
                  nc.vector.reciprocal(rtr[:], tre[:])
                  coh = o_new()
                  nc.gpsimd.tensor_mul(out=coh[:], in0=lam[:], in1=rtr[:])
                  dma_out('coherence', coh[:])
                  aniso = o_new()
                  nc.gpsimd.tensor_mul(out=aniso[:], in0=dd[:], in1=rtr[:])
                  dma_out('anisotropy', aniso[:])

                  # ---- residual / checker / stripe energies ----
                  o_copy_out('res_eh', convB(rhsq))
                  o_copy_out('res_ev', convB(rvsq))
                  o_copy_out('chk_e', convB(chksq), eng='s')
                  o_copy_out('str_e', convB(s2), eng='s')

                  # ---- simple O-grid convs ----
                  o_copy_out('lap', convO('lap'))
                  hxx_ps = convO('hxx')
                  abshxx = o_new()
                  nc.scalar.activation(h3(abshxx[:]), ps3(hxx_ps, NO), AF.Abs)
                  o_copy_out('hxx', hxx_ps)
                  hyy_ps = convO('hyy')
                  abshyy = o_new()
                  nc.scalar.activation(h3(abshyy[:]), ps3(hyy_ps, NO), AF.Abs)
                  o_copy_out('hyy', hyy_ps)
                  dsd = o_new()
                  nc.gpsimd.tensor_sub(out=dsd[:], in0=abshxx[:], in1=abshyy[:])
                  dma_out('dsd', dsd[:])
                  o_copy_out('hxy', convO('hxy'))
                  o_copy_out('gdm', convO('gdm'))
                  o_copy_out('gda', convO('gda'), eng='s')
                  o_copy_out('hsh', convO('hsh'), eng='s')
                  o_copy_out('hsv', convO('hsv'), eng='s')
                  o_copy_out('g45', convO('g45'))
                  o_copy_out('g135', convO('g135'))
                  o_copy_out('dctp', convO('dct'), eng='s')

                  # ---- hgh / hgv / mhc ----
                  hgh_ps = convO('hgh')
                  hghS = o_copy_out('hgh', hgh_ps, eng='s')
                  hgv_ps = convO('hgv')
                  hgvS = o_copy_out('hgv', hgv_ps, eng='s')
                  hd = o_new()
                  nc.vector.tensor_sub(out=h3(hd[:]), in0=ps3(hgh_ps, NO), in1=h3(hgvS[:]))
                  had = o_new()
                  nc.scalar.activation(had[:], hd[:], AF.Abs)
                  dma_out('ha_dis', had[:])

                  mhcf_ps = convO('mhcf')
                  bmf = o_new()
                  nc.vector.tensor_sub(out=h3(bmf[:]), in0=h3(bayerO), in1=ps3(mhcf_ps, NO))
                  gbm = o_new()
                  nc.gpsimd.tensor_mul(out=gbm[:], in0=bmf[:], in1=gmask_t[:])
                  mhc = o_new()
                  nc.vector.tensor_add(out=h3(mhc[:]), in0=ps3(mhcf_ps, NO), in1=h3(gbm[:]))
                  dma_out('mhc', mhc[:])
                  mha1 = o_new()
                  nc.vector.scalar_tensor_tensor(out=mha1[:], in0=hghS[:], scalar=-0.5,
                                                 in1=mhc[:], op0=A.mult, op1=A.add)
                  mhc_ha = o_new()
                  nc.vector.scalar_tensor_tensor(out=mhc_ha[:], in0=hgvS[:], scalar=-0.5,
                                                 in1=mha1[:], op0=A.mult, op1=A.add)
                  dma_out('mhc_ha', mhc_ha[:])
                  rres = o_new()
                  nc.gpsimd.tensor_mul(out=rres[:], in0=bmf[:], in1=rbmask_t[:, 0:W])
                  dma_out('rres', rres[:])
                  bres = o_new()
                  nc.gpsimd.tensor_mul(out=bres[:], in0=bmf[:], in1=rbmask_t[:, W:2 * W])
                  dma_out('bres', bres[:])

                  # ---- smooth5 fills (polyphase, 1/den folded into bands) ----
                  strip_pp = strip[:].rearrange("k (c t) -> k c t", t=2)

                  def sm5_conv(groups):
                      # groups: list of (passes, cX); all accumulate into one psum
                      ps = ppo.tile([MO, 1024], f32, tag="po", name="ps")
                      for p in range(2):
                          chain = [(bi, j0, cX) for passes, cX in groups
                                   for (pp_, bi, j0) in passes if pp_ == p]
                          for i, (bi, j0, cX) in enumerate(chain):
                              nc.tensor.matmul(
                                  ps[:, p * 512:p * 512 + NO],
                                  bandO(bi), strip_pp[:, j0:j0 + NO, cX],
                                  start=(i == 0), stop=(i == len(chain) - 1))
                      return ps

                  def ppv(t):
                      # [MO, W] tile viewed phase-major: [MO, 2, 384], col = 2*j + p
                      return t[:].rearrange("p (j t) -> p t j", t=2)

                  gf_ps = sm5_conv([(P_SM5_GF[0], 1), (P_SM5_GF[1], 0)])
                  gfS = o_new()
                  nc.vector.tensor_copy(out=h3(gfS[:]), in_=ps3(gf_ps, NO))
                  gbf_ps = sm5_conv([P_SM5['gbf']])
                  gbfS = o_new()
                  nc.vector.tensor_copy(out=h3(gbfS[:]), in_=ps3(gbf_ps, NO))
                  rf_ps = sm5_conv([P_SM5['rf']])
                  rg = o_new()
                  nc.vector.tensor_sub(out=ppv(rg), in0=ps3(rf_ps, NO), in1=h3(gfS[:]))
                  dma_out('rg', rg[:])
                  bf_ps = sm5_conv([P_SM5['bf']])
                  bg = o_new()
                  nc.vector.tensor_sub(out=ppv(bg), in0=ps3(bf_ps, NO), in1=h3(gfS[:]))
                  dma_out('bg', bg[:])
                  grf_ps = sm5_conv([P_SM5['grf']])
                  gpd = o_new()
                  nc.vector.tensor_sub(out=ppv(gpd), in0=ps3(grf_ps, NO), in1=h3(gbfS[:]))
                  dma_out('gpd', gpd[:])

                  # ---- line variances ----
                  mh_ps = conv_split('avgh5_h', 'avgh5_l', xh_t, xl_t)
                  mh2 = o_new()
                  nc.scalar.activation(h3(mh2[:]), ps3(mh_ps, NO), AF.Square)
                  qh_ps = conv_split('avgh5_h', 'avgh5_l', bsqr, bsql)
                  lvh = o_new()
                  nc.vector.scalar_tensor_tensor(out=h3(lvh[:]), in0=h3(mh2[:]),
                                                 scalar=-1.0, in1=ps3(qh_ps, NO),
                                                 op0=A.mult, op1=A.add)
                  dma_out('lvh', lvh[:])
                  mv_ps = conv_split('avgv5_h', 'avgv5_l', xh_t, xl_t)
                  mv2 = o_new()
                  nc.scalar.activation(h3(mv2[:]), ps3(mv_ps, NO), AF.Square)
                  qv_ps = conv_split('avgv5_h', 'avgv5_l', bsqr, bsql)
                  lvv = o_new()
                  nc.vector.scalar_tensor_tensor(out=h3(lvv[:]), in0=h3(mv2[:]),
                                                 scalar=-1.0, in1=ps3(qv_ps, NO),
                                                 op0=A.mult, op1=A.add)
                  dma_out('lvv', lvv[:])
                  lvd = o_new()
                  nc.gpsimd.tensor_sub(out=lvd[:], in0=lvh[:], in1=lvv[:])
                  dma_out('lvd', lvd[:])
                  alvd = o_new()
                  nc.scalar.activation(alvd[:], lvd[:], AF.Abs)
                  den2 = o_new()
                  nc.gpsimd.tensor_add(out=den2[:], in0=lvh[:], in1=lvv[:])
                  d2e = o_new()
                  nc.vector.tensor_scalar_add(d2e[:], den2[:], EPS)
                  rden2 = o_new()
                  nc.vector.reciprocal(rden2[:], d2e[:])
                  dconf = o_new()
                  nc.gpsimd.tensor_mul(out=dconf[:], in0=alvd[:], in1=rden2[:])
                  dma_out('dconf', dconf[:])

                  # ---- local mean / variance ----
                  lm_ps = convO('box3')
                  lmS = o_copy_out('lmean', lm_ps, eng='s')
                  lm2 = o_new()
                  nc.scalar.activation(lm2[:], lmS[:], AF.Square)
                  lq_ps = convO('box3', rhs=bsqr)
                  lvar = o_new()
                  nc.vector.scalar_tensor_tensor(out=h3(lvar[:]), in0=h3(lm2[:]),
                                                 scalar=-1.0, in1=ps3(lq_ps, NO),
                                                 op0=A.mult, op1=A.add)
                  dma_out('lvar', lvar[:])

    nc.compile()
    return nc


def _get_program(loop=1, timing=False):
    key = (loop, timing)
    if key not in _PROGRAM:
        _PROGRAM[key] = _build_program(loop, timing)
    return _PROGRAM[key]


def _host_constants():
    def kmajor(b):
        n, k, m = b.shape
        return np.ascontiguousarray(np.transpose(b, (1, 0, 2)).reshape(k, n * m))

    consts = {
        "bandsO": kmajor(BANDS_O),
        "bandsE": kmajor(BANDS_E),
        "bandsB": kmajor(BANDS_B),
    }
    # strip row k <-> image row (k - 3): parity (k+1)%2
    # strip col c <-> image col (c - 6): parity (c)%2
    rb = np.zeros((MO, 2 * W), np.float32)
    rb[:, 0:W] = _tile_pattern(_mask_pattern('r'), MO, W)
    rb[:, W:2 * W] = _tile_pattern(_mask_pattern('b'), MO, W)
    consts["rbmask"] = rb
    consts["gmask"] = _tile_pattern(_mask_pattern('g'), MO, W)
    m7 = np.zeros((7, MO, W), np.float32)
    for i, nm in enumerate(['r', 'g', 'b', 'gr', 'gb', 'row', 'col']):
        m7[i] = _tile_pattern(_mask_pattern(nm), MO, W)
    consts["masks7"] = m7
    return consts


def _in_maps(bayer):
    consts = _host_constants()

    def kmajor(bnd):
        n, k, mm = bnd.shape
        return np.ascontiguousarray(np.transpose(bnd, (1, 0, 2)).reshape(k, n * mm))

    padded = np.pad(bayer[:, 0], ((0, 0), (PH, PH), (PW, PW)), mode='reflect')
    in_maps = []
    for c in range(NCORES):
        b, j = divmod(c, CORES_PER_BATCH)
        strip = padded[b, j * RPC: j * RPC + SR, :]
        m = dict(consts)
        m["xs"] = np.ascontiguousarray(strip)
        if j == 0 or j == CORES_PER_BATCH - 1:
            bb = np.stack([_BB_STD,
                           _bb_variant('top') if j == 0 else _BB_STD,
                           _bb_variant('bot') if j == CORES_PER_BATCH - 1 else _BB_STD])
            m["bandsB"] = kmajor(bb)
        in_maps.append(m)
    return in_maps


def kernel(bayer: np.ndarray) -> np.ndarray:
    from concourse.bass_utils import run_bass_kernel_spmd

    bayer = np.asarray(bayer, np.float32)
    assert bayer.shape == (B, 1, H, W), bayer.shape
    nc = _get_program()
    res = run_bass_kernel_spmd(nc, _in_maps(bayer), list(range(NCORES)))
    out = np.zeros((B, 52, H, W), np.float32)
    for c in range(NCORES):
        b, j = divmod(c, CORES_PER_BATCH)
        out[b, :, j * RPC:(j + 1) * RPC, :] = res.results[c]["out"]
    return out



# revision 15
# speedup vs baseline: 1.3622x; 1.3622x over previous
"""Trainium2 Bass kernel for nn_BayerFeatureExtractor.

Computes 52 feature channels from a [2,1,768,768] bayer image, data-parallel
over 8 NeuronCores (each core: one batch image x 192 rows, 2 row-blocks).

Strategy:
  - Host reflect-pads each batch image by (3 rows, 6 cols); each core gets a
    [198, 780] fp32r strip (rows on SBUF partitions).
  - All convolutions run on the TensorEngine as banded matmuls (fp32r, full
    rate): contraction over input rows with a banded weight matrix encoding
    the kernel's row profile; one PSUM-accumulated pass per nonzero kernel
    column, with the moving operand shifted along the free (column) dim.
  - Intermediates that feed 3x3 box filters are computed on an "E" grid
    extended by 1 row/col so the second conv stage needs no partition-offset
    reads (compute engines require partition start 0); reflect behavior of
    intermediates at image borders is exact because every relevant kernel
    profile is symmetric (or enters squared).
  - Pointwise math spread across Vector (DVE), Scalar (ACT), GPSIMD engines.
  - Morphological gradient via 3 row-shifted DMA copies + max/min chains.
"""
import sys
import math

sys.path.insert(0, '/opt/trn_rl_repo')

import numpy as np

EPS = 1e-6

H = 768
W = 768
B = 2
NCORES = 8
CORES_PER_BATCH = 4
RPC = H // CORES_PER_BATCH          # 192 output rows per core
NBLK = 2
BR = RPC // NBLK                    # 96 output rows per block
PH = 3                              # host row padding
PW = 6                              # host col padding
SR = RPC + 2 * PH                   # 198 strip rows
SW = W + 2 * PW                     # 780 strip cols
KIN = BR + 2 * PH                   # 102 contraction rows per block
ME, MO = 98, 96                     # E-grid / O-grid matmul M
NE, NO = 386, 384                   # matmul half widths
EW = 772                            # E tile width (covers out cols -2..769)


# ---------------------------------------------------------------- kernels ---
def _npk(a, s=1.0):
    return np.asarray(a, dtype=np.float32) * np.float32(s)


def _gabor(theta, sigma=1.1, lambd=3.0, gamma=0.65):
    c = np.arange(-2, 3, dtype=np.float32)
    yy, xx = np.meshgrid(c, c, indexing='ij')
    xt = xx * math.cos(theta) + yy * math.sin(theta)
    yt = -xx * math.sin(theta) + yy * math.cos(theta)
    k = np.exp(-(xt ** 2 + gamma ** 2 * yt ** 2) / (2.0 * sigma ** 2)) * np.cos(
        2.0 * math.pi * xt / lambd)
    k = k - k.mean()
    return (k / max(np.abs(k).sum(), 1e-6)).astype(np.float32)


def _dct_like(u=2, v=2, size=5):
    c = np.arange(size, dtype=np.float32)
    yy, xx = np.meshgrid(c, c, indexing='ij')
    k = np.cos(math.pi * (2 * xx + 1) * u / (2 * size)) * np.cos(
        math.pi * (2 * yy + 1) * v / (2 * size))
    k = k - k.mean()
    return (k / max(np.abs(k).sum(), 1e-6)).astype(np.float32)


K_LAP = _npk([[0, 1, 0], [1, -4, 1], [0, 1, 0]])
K_HXX = _npk([[1, -2, 1]])                     # row kernel
K_HYY = _npk([[1], [-2], [1]])                 # col kernel
K_HXY = _npk([[1, 0, -1], [0, 0, 0], [-1, 0, 1]], 0.25)
K_GX = _npk([[-1, 0, 1], [-2, 0, 2], [-1, 0, 1]], 0.125)
K_GY = _npk([[-1, -2, -1], [0, 0, 0], [1, 2, 1]], 0.125)
K_GDM = _npk([[-2, -1, 0], [-1, 0, 1], [0, 1, 2]], 0.125)
K_GDA = _npk([[0, 1, 2], [-1, 0, 1], [-2, -1, 0]], 0.125)
K_CHK = _npk([[1, -1, 1], [-1, 1, -1], [1, -1, 1]], 1.0 / 9.0)

HGH_V = np.array([-0.25, 0.5, 0.5, 0.5, -0.25], np.float32)
K_HSH = _npk([[-0.5, 0.0, 1.0, 0.0, -0.5]])
K_HSV = _npk([[-0.5], [0.0], [1.0], [0.0], [-0.5]])
K_HGH = HGH_V.reshape(1, 5)
K_HGV = HGH_V.reshape(5, 1)
K_MHC = _npk([[0, 0, -1, 0, 0], [0, 0, 2, 0, 0], [-1, 2, 4, 2, -1],
              [0, 0, 2, 0, 0], [0, 0, -1, 0, 0]], 0.125)
K_STX = _npk([[0.25, -1.0, 1.5, -1.0, 0.25]])
K_STY = K_STX.reshape(5, 1).copy()
K_G45 = _gabor(math.pi / 4.0)
K_G135 = _gabor(3.0 * math.pi / 4.0)
K_DCT = _dct_like()
K_SMOOTH5 = (_npk([[1, 2, 3, 2, 1], [2, 4, 6, 4, 2], [3, 6, 9, 6, 3],
                   [2, 4, 6, 4, 2], [1, 2, 3, 2, 1]]) / np.float32(81.0))
K_RESH = (np.eye(1, 5, 2, dtype=np.float32) - K_HGH)      # delta - hgh (1x5)
K_RESV = (np.eye(5, 1, -2, dtype=np.float32) - K_HGV)     # delta - hgv (5x1)
K_BOX3 = np.full((3, 3), 1.0 / 9.0, np.float32)
K_AVGH5 = np.full((1, 5), 0.2, np.float32)
K_AVGV5 = np.full((5, 1), 0.2, np.float32)


def _mask_pattern(name):
    # value at (row parity, col parity), gbrg pattern
    m = np.zeros((2, 2), np.float32)
    if name == 'r':
        m[1, 0] = 1.0
    elif name == 'b':
        m[0, 1] = 1.0
    elif name == 'gr':
        m[1, 1] = 1.0
    elif name == 'gb':
        m[0, 0] = 1.0
    elif name == 'g':
        m[0, 0] = 1.0; m[1, 1] = 1.0
    elif name == 'row':
        m[1, :] = 1.0
    elif name == 'col':
        m[:, 1] = 1.0
    return m


def _den_pattern(name):
    # conv(mask, SMOOTH5) is exactly 2x2-periodic (reflect == parity ext.)
    pat = _mask_pattern(name)
    g = np.zeros((16, 16), np.float32)
    for r in range(16):
        for c in range(16):
            g[r, c] = pat[r % 2, c % 2]
    out = np.zeros((2, 2), np.float32)
    for r in (6, 7):
        for c in (6, 7):
            acc = np.float32(0.0)
            for dy in range(5):
                for dx in range(5):
                    acc += K_SMOOTH5[dy, dx] * g[r + dy - 2, c + dx - 2]
            out[r % 2, c % 2] = acc
    return np.maximum(out, EPS)


def _tile_pattern(pat, rows, cols, row_shift=0, col_shift=0):
    out = np.zeros((rows, cols), np.float32)
    for rp in range(2):
        for cp in range(2):
            out[rp::2, cp::2] = pat[(rp + row_shift) % 2, (cp + col_shift) % 2]
    return out


# ------------------------------------------------------------ band builder ---
class Bands:
    """Dedup banded lhsT matrices per grid ('O' out rows, 'E' extended, 'B' box)."""

    def __init__(self):
        self.items = {'O': [], 'E': [], 'B': []}
        self.index = {}

    def get(self, grid, prof):
        key = (grid, tuple(np.round(np.asarray(prof, np.float64), 10)))
        if key in self.index:
            return self.index[key]
        prof = np.asarray(prof, np.float32)
        kh = len(prof)
        off = kh // 2
        if grid == 'O':
            m = np.zeros((KIN, MO), np.float32)
            for mm in range(MO):
                base = mm + PH - off
                for t in range(kh):
                    m[base + t, mm] = prof[t]
        elif grid == 'E':
            m = np.zeros((KIN, ME), np.float32)
            for ii in range(ME):
                base = ii + PH - 1 - off
                for t in range(kh):
                    m[base + t, ii] = prof[t]
        else:  # 'B': 3-row box applied to E tiles
            m = np.zeros((ME, MO), np.float32)
            for mm in range(MO):
                for t in range(kh):
                    m[mm + t, mm] = prof[t]
        idx = len(self.items[grid])
        self.items[grid].append(m)
        self.index[key] = idx
        return idx

    def passes(self, grid, K):
        K = np.atleast_2d(np.asarray(K, np.float32))
        kw = K.shape[1]
        out = []
        for dxi in range(kw):
            col = K[:, dxi]
            if np.any(col != 0.0):
                out.append((dxi - kw // 2, self.get(grid, col)))
        return out


_BANDS = Bands()

P_E = {
    'gx': _BANDS.passes('E', K_GX),
    'gy': _BANDS.passes('E', K_GY),
    'chk': _BANDS.passes('E', K_CHK),
    'stx': _BANDS.passes('E', K_STX),
    'sty': _BANDS.passes('E', K_STY),
    'resh': _BANDS.passes('E', K_RESH),
    'resv': _BANDS.passes('E', K_RESV),
}
P_O = {
    'lap': _BANDS.passes('O', K_LAP),
    'hxx': _BANDS.passes('O', K_HXX),
    'hyy': _BANDS.passes('O', K_HYY),
    'hxy': _BANDS.passes('O', K_HXY),
    'gdm': _BANDS.passes('O', K_GDM),
    'gda': _BANDS.passes('O', K_GDA),
    'hsh': _BANDS.passes('O', K_HSH),
    'hsv': _BANDS.passes('O', K_HSV),
    'hgh': _BANDS.passes('O', K_HGH),
    'hgv': _BANDS.passes('O', K_HGV),
    'mhcf': _BANDS.passes('O', K_MHC),
    'g45': _BANDS.passes('O', K_G45),
    'g135': _BANDS.passes('O', K_G135),
    'dct': _BANDS.passes('O', K_DCT),
    'sm5': _BANDS.passes('O', K_SMOOTH5),
    'avgh5': _BANDS.passes('O', K_AVGH5),
    'avgv5': _BANDS.passes('O', K_AVGV5),
    'box3': _BANDS.passes('O', K_BOX3),
}
P_B = _BANDS.passes('B', K_BOX3)


BANDS_E = np.stack(_BANDS.items['E'])          # [nE, 102, 98]
def _trunc10(a):
    b = np.asarray(a, np.float32).copy()
    v = b.view(np.uint32)
    v &= np.uint32(0xFFFFE000)
    return b


K_AVGH5_H = _trunc10(K_AVGH5)
K_AVGH5_L = K_AVGH5 - K_AVGH5_H
K_AVGV5_H = _trunc10(K_AVGV5)
K_AVGV5_L = K_AVGV5 - K_AVGV5_H
P_O.update({
    'avgv5_h': _BANDS.passes('O', K_AVGV5_H),
    'avgv5_l': _BANDS.passes('O', K_AVGV5_L),
})

# --- polyphase smooth5 bands: row-parity mask and 1/den folded into bands.
# For mask X (row parity rX, col parity cX) and den channel D, out-col phase
# p: passes use only taps dx with (p+dx)%2 == cX; band rows of wrong parity
# are zeroed; band cols scaled by 1/den_D(out-row parity, p).
_SM5_META = {'r': (1, 0), 'b': (0, 1), 'gr': (1, 1), 'gb': (0, 0)}


def _sm5_passes(X, D):
    rX, cX = _SM5_META[X]
    rden = 1.0 / _den_pattern(D)
    out = []
    for p in range(2):
        for dxi in range(5):
            if (p + dxi) % 2 != cX:
                continue
            m = np.zeros((KIN, MO), np.float32)
            for mm in range(MO):
                for t in range(5):
                    k = mm + 1 + t
                    if (k + 1) % 2 == rX:
                        m[k, mm] = K_SMOOTH5[t, dxi] * rden[mm % 2, p]
            key = ('SM5', X, D, p, dxi)
            if key not in _BANDS.index:
                _BANDS.index[key] = len(_BANDS.items['O'])
                _BANDS.items['O'].append(m)
            # rhs strided-view offset (in cX-parity column units)
            j0 = (PW + p + (dxi - 2) - cX) // 2
            out.append((p, _BANDS.index[key], j0))
    return out


P_SM5 = {
    'rf': (_sm5_passes('r', 'r'), 0),
    'bf': (_sm5_passes('b', 'b'), 1),
    'grf': (_sm5_passes('gr', 'gr'), 1),
    'gbf': (_sm5_passes('gb', 'gb'), 0),
}
P_SM5_GF = (_sm5_passes('gr', 'g'), _sm5_passes('gb', 'g'))  # accumulate both

BANDS_O = np.stack(_BANDS.items['O'])          # [nO, 102, 96]
_BB_STD = _BANDS.items['B'][0]
BANDS_B = np.stack([_BB_STD, _BB_STD, _BB_STD])   # [3, 98, 96]: std, top-slot, bot-slot


def _bb_variant(kind):
    m = _BB_STD.copy()
    if kind == 'top':
        m[0, 0] = -m[0, 0]
    else:
        m[ME - 1, MO - 1] = -m[ME - 1, MO - 1]
    return m

CH = {n: i for i, n in enumerate([
    'r', 'g', 'b', 'gr', 'gb', 'rowm', 'colm',
    'lap', 'hxx', 'hyy', 'hxy', 'mgrad', 'gx', 'gy', 'gdm', 'gda', 'gmag',
    'coherence', 'anisotropy', 'hsh', 'hsv', 'hgh', 'hgv', 'ha_dis',
    'res_h', 'res_v', 'res_eh', 'res_ev', 'dgd', 'dsd', 'lvh', 'lvv', 'lvd',
    'dconf', 'rg', 'bg', 'gpd', 'mhc', 'mhc_ha', 'rres', 'bres',
    'stx', 'sty', 'chk', 'g45', 'g135', 'dctp', 'chk_e', 'str_e', 'lmean',
    'lvar', 'gen'])}


# ------------------------------------------------------------- bass program ---
_PROGRAM = {}


def _build_program(loop=1, timing=False):
    import concourse.bacc as bacc
    import concourse.mybir as mybir
    from concourse.tile import TileContext

    f32 = mybir.dt.float32
    f32r = mybir.dt.float32r
    u8 = mybir.dt.uint8
    A = mybir.AluOpType
    AF = mybir.ActivationFunctionType

    nc = bacc.Bacc("TRN2")

    if timing:
        def declare(name, shape, dtype, isOutput):
            return nc.dram_tensor(name, shape, dtype).ap()
        tin = nc.declare_dram_parameter("tin", [1, 4], mybir.dt.float32, isOutput=False)
        tout = nc.declare_dram_parameter("tout", [1, 4], mybir.dt.float32, isOutput=True)
    else:
        def declare(name, shape, dtype, isOutput):
            return nc.declare_dram_parameter(name, shape, dtype, isOutput=isOutput)

    def register_const(value):
        t = nc.alloc_sbuf_tensor(f"constf32-{value}", [128, 1], f32)
        nc.gpsimd.memset(t.ap(), value)
        nc.const_aps.aps[(f32, value)] = t.ap()

    register_const(EPS)
    nc.all_engine_barrier()

    nO, nE, nB = BANDS_O.shape[0], BANDS_E.shape[0], BANDS_B.shape[0]
    xs_ext = declare("xs", [SR, SW], f32r, isOutput=False)
    bo_ext = declare("bandsO", [KIN, nO * MO], f32r, isOutput=False)
    be_ext = declare("bandsE", [KIN, nE * ME], f32r, isOutput=False)
    bb_ext = declare("bandsB", [ME, nB * MO], f32r, isOutput=False)
    rbmask_ext = declare("rbmask", [MO, 2 * W], f32, isOutput=False)
    gmask_ext = declare("gmask", [MO, W], f32, isOutput=False)
    masks7_ext = declare("masks7", [7, MO, W], f32, isOutput=False)
    out_ext = declare("out", [52, RPC, W], f32, isOutput=True)

    with TileContext(nc) as tc:
        with (
            tc.tile_pool(name="const", bufs=1) as cpool,
            tc.tile_pool(name="work", bufs=1) as wpool,
            tc.tile_pool(name="ebuf", bufs=8) as epool,
            tc.tile_pool(name="obuf", bufs=15) as opool,
            tc.tile_pool(name="psE", bufs=2, space="PSUM") as ppe,
            tc.tile_pool(name="psO", bufs=2, space="PSUM") as ppo,
        ):
            # ---- constants -> SBUF (once) ----
            bo_t = cpool.tile([KIN, nO * MO], f32r)
            nc.sync.dma_start(out=bo_t[:], in_=bo_ext[:])
            be_t = cpool.tile([KIN, nE * ME], f32r)
            nc.sync.dma_start(out=be_t[:], in_=be_ext[:])
            bb_t = cpool.tile([ME, nB * MO], f32r)
            nc.sync.dma_start(out=bb_t[:], in_=bb_ext[:])
            rbmask_t = cpool.tile([MO, 2 * W], f32)
            nc.sync.dma_start(out=rbmask_t[:], in_=rbmask_ext[:])
            gmask_t = cpool.tile([MO, W], f32)
            nc.sync.dma_start(out=gmask_t[:], in_=gmask_ext[:])

            def bandO(i):
                return bo_t[:, i * MO:(i + 1) * MO]

            def bandE(i):
                return be_t[:, i * ME:(i + 1) * ME]

            def bandB(i):
                return bb_t[:, i * MO:(i + 1) * MO]

            def h3(ap):
                # [P, 2N] (possibly offset slice) -> [P, 2, N]
                return ap.rearrange("p (b n) -> p b n", b=2)

            if timing:
                nc.sync.dma_start(out=tout[:], in_=tin[:])
            # mask output channels: DRAM -> DRAM
            for ch_i, name in enumerate(['r', 'g', 'b', 'gr', 'gb', 'rowm', 'colm']):
                for blk in range(NBLK):
                    nc.sync.dma_start(out=out_ext[CH[name], blk * BR:(blk + 1) * BR, :],
                                      in_=masks7_ext[ch_i])

            import contextlib
            loop_cm = tc.For_i(0, loop, 1) if loop > 1 else contextlib.nullcontext()
            with loop_cm:
              for blk in range(NBLK):
                  r0 = blk * BR

                  # ---- inputs ----
                  strip = wpool.tile([KIN, SW], f32r, tag="strip", bufs=2)
                  nc.scalar.dma_start(out=strip[:], in_=xs_ext[r0:r0 + KIN, :])
                  stripf = strip[:].bitcast(f32)
                  T0 = wpool.tile([MO, SW], f32, tag="T0", bufs=2)
                  T1 = wpool.tile([MO, SW], f32, tag="T1", bufs=2)
                  T2 = wpool.tile([MO, SW], f32, tag="T2", bufs=2)
                  nc.scalar.dma_start(out=T0[:], in_=xs_ext[r0 + 2:r0 + 2 + MO, :].bitcast(f32))
                  nc.scalar.dma_start(out=T1[:], in_=xs_ext[r0 + 3:r0 + 3 + MO, :].bitcast(f32))
                  nc.scalar.dma_start(out=T2[:], in_=xs_ext[r0 + 4:r0 + 4 + MO, :].bitcast(f32))

                  bayerO = T1[:, PW:PW + W]

                  # ---- derived matmul inputs ----
                  # strip32 load dropped: strip f32r bytes ARE the f32 values
                  bsq32 = wpool.tile([KIN, SW], f32, tag="bsq32")
                  nc.scalar.activation(bsq32[:], stripf, AF.Square)
                  # hi/lo fp32r split of bayer and bayer^2 for full-precision
                  # variance convs at fp32r matmul rate
                  xh_t = wpool.tile([KIN, SW], f32r, tag="xh")
                  nc.scalar.copy(out=xh_t[:], in_=stripf)
                  xl_t = wpool.tile([KIN, SW], f32r, tag="xl")
                  nc.vector.tensor_sub(out=xl_t[:], in0=stripf, in1=xh_t[:].bitcast(f32))
                  bsqr = wpool.tile([KIN, SW], f32r, tag="bsqr")
                  nc.scalar.activation(bsqr[:], stripf, AF.Square)
                  bsql = wpool.tile([KIN, SW], f32r, tag="bsql")
                  nc.vector.tensor_sub(out=bsql[:], in0=bsq32[:], in1=bsqr[:].bitcast(f32))

                  # ---- conv helpers ----
                  def conv(passes, grid, rhs, band_fn, M, NH):
                      pool = ppe if grid == 'E' else ppo
                      ps = pool.tile([M, 1024], f32, tag="pe" if grid == 'E' else "po",
                                     name="ps")
                      shift = {'E': PW - 2, 'O': PW, 'B': 2}[grid]
                      for h in range(2):
                          for i, (dx, bi) in enumerate(passes):
                              nc.tensor.matmul(
                                  ps[:, h * 512:h * 512 + NH],
                                  band_fn(bi),
                                  rhs[:, shift + dx + h * NH: shift + dx + h * NH + NH],
                                  start=(i == 0), stop=(i == len(passes) - 1))
                      return ps

                  def convE(name, rhs=None):
                      return conv(P_E[name], 'E', (rhs if rhs is not None else strip)[:], bandE, ME, NE)

                  def convO(name, rhs=None):
                      return conv(P_O[name], 'O', (rhs if rhs is not None else strip)[:], bandO, MO, NO)

                  def convB(rhs_tile):
                      return conv(P_B, 'B', rhs_tile[:ME], bandB, MO, NO)

                  def conv_split(kh_name, kl_name, xh, xl):
                      ps = ppo.tile([MO, 1024], f32, tag="po", name="ps")
                      for h in range(2):
                          chain = ([(bi, xh, dx) for dx, bi in P_O[kh_name]]
                                   + [(bi, xl, dx) for dx, bi in P_O[kh_name]]
                                   + [(bi, xh, dx) for dx, bi in P_O[kl_name]])
                          for i, (bi, rhs_t, dx) in enumerate(chain):
                              nc.tensor.matmul(
                                  ps[:, h * 512:h * 512 + NO],
                                  bandO(bi),
                                  rhs_t[:, PW + dx + h * NO: PW + dx + h * NO + NO],
                                  start=(i == 0), stop=(i == len(chain) - 1))
                      return ps


                  def ps3(ps, NH):
                      return h3(ps[:, 0:1024])[:, :, 0:NH]

                  def e_named(tag, dtype=f32):
                      return wpool.tile([ME, EW], dtype, tag=tag, name=tag)

                  def e_roll(dtype=f32):
                      return epool.tile([ME, EW], dtype, tag="ebuf", name="eb")

                  def o_new():
                      return opool.tile([MO, W], f32, tag="obuf", name="ob")

                  def dma_out(name, ap):
                      nc.sync.dma_start(out=out_ext[CH[name], r0:r0 + BR, :], in_=ap)

                  def o_copy_out(name, ps, eng='v'):
                      t = o_new()
                      if eng == 'v':
                          nc.vector.tensor_copy(out=h3(t[:]), in_=ps3(ps, NO))
                      else:
                          nc.scalar.copy(out=h3(t[:]), in_=ps3(ps, NO))
                      dma_out(name, t[:])
                      return t

                  def e_win(t):
                      return t[1:97, 2:770]

                  # ---- E-grid stage 1 + products ----
                  gx_ps = convE('gx')
                  gy_ps = convE('gy')
                  gyS = e_named("gyS")
                  nc.vector.tensor_copy(out=h3(gyS[:]), in_=ps3(gy_ps, NE))
                  dma_out('gy', e_win(gyS))
                  gxx = e_named("gxx", f32r)
                  nc.scalar.activation(h3(gxx[:]), ps3(gx_ps, NE), AF.Square)
                  gyy = e_named("gyy", f32r)
                  nc.scalar.activation(gyy[:], gyS[:], AF.Square)
                  gxy = e_named("gxy", f32r)
                  nc.vector.tensor_mul(out=h3(gxy[:]), in0=ps3(gx_ps, NE), in1=h3(gyS[:]))
                  # virtual cols -1/768 of the gx*gy product have flipped sign
                  # relative to the reflect of the product; fix before the box.
                  nc.vector.tensor_scalar_mul(out=gxy[:, 1:2], in0=gxy[:, 1:2], scalar1=-1.0)
                  nc.vector.tensor_scalar_mul(out=gxy[:, 770:771], in0=gxy[:, 770:771], scalar1=-1.0)
                  absgx = e_roll()
                  nc.scalar.activation(h3(absgx[:]), ps3(gx_ps, NE), AF.Abs)
                  gxS = e_roll()
                  nc.scalar.copy(out=h3(gxS[:]), in_=ps3(gx_ps, NE))
                  dma_out('gx', e_win(gxS))
                  absgy = e_roll()
                  nc.scalar.activation(absgy[:], gyS[:], AF.Abs)
                  dgd = e_roll()
                  nc.gpsimd.tensor_sub(out=dgd[:], in0=absgx[:], in1=absgy[:])
                  dma_out('dgd', e_win(dgd))
                  g2 = e_roll()
                  nc.gpsimd.tensor_add(out=g2[:], in0=gxx[:].bitcast(f32), in1=gyy[:].bitcast(f32))
                  gmag = e_roll()
                  nc.scalar.activation(gmag[:], g2[:], AF.Sqrt, bias=EPS)
                  dma_out('gmag', e_win(gmag))

                  chk_ps = convE('chk')
                  chksq = e_named("chksq", f32r)
                  nc.scalar.activation(h3(chksq[:]), ps3(chk_ps, NE), AF.Square)
                  chkS = e_roll()
                  nc.scalar.copy(out=h3(chkS[:]), in_=ps3(chk_ps, NE))
                  dma_out('chk', e_win(chkS))

                  stx_ps = convE('stx')
                  stxsq = e_named("stxsq")
                  nc.scalar.activation(h3(stxsq[:]), ps3(stx_ps, NE), AF.Square)
                  stxS = e_roll()
                  nc.scalar.copy(out=h3(stxS[:]), in_=ps3(stx_ps, NE))
                  dma_out('stx', e_win(stxS))
                  sty_ps = convE('sty')
                  stysq = e_named("stysq")
                  nc.scalar.activation(h3(stysq[:]), ps3(sty_ps, NE), AF.Square)
                  styS = e_roll()
                  nc.vector.tensor_copy(out=h3(styS[:]), in_=ps3(sty_ps, NE))
                  dma_out('sty', e_win(styS))
                  s2 = e_named("s2", f32r)
                  nc.gpsimd.tensor_add(out=s2[:], in0=stxsq[:], in1=stysq[:])

                  resh_ps = convE('resh')
                  rhsq = e_named("rhsq", f32r)
                  nc.scalar.activation(h3(rhsq[:]), ps3(resh_ps, NE), AF.Square)
                  reshS = e_roll()
                  nc.vector.tensor_copy(out=h3(reshS[:]), in_=ps3(resh_ps, NE))
                  dma_out('res_h', e_win(reshS))
                  resv_ps = convE('resv')
                  rvsq = e_named("rvsq", f32r)
                  nc.scalar.activation(h3(rvsq[:]), ps3(resv_ps, NE), AF.Square)
                  resvS = e_roll()
                  nc.vector.tensor_copy(out=h3(resvS[:]), in_=ps3(resv_ps, NE))
                  dma_out('res_v', e_win(resvS))

                  # ---- mgrad (GPSIMD) ----
                  v1 = wpool.tile([MO, SW], f32, tag="mgtmp")
                  nc.vector.tensor_max(out=v1[:], in0=T0[:], in1=T2[:])
                  v3 = wpool.tile([MO, SW], f32, tag="v3")
                  nc.vector.tensor_max(out=v3[:], in0=v1[:], in1=T1[:])
                  n1 = wpool.tile([MO, SW], f32, tag="mgtmp")
                  nc.vector.tensor_tensor(out=n1[:], in0=T0[:], in1=T2[:], op=A.min)
                  n3 = wpool.tile([MO, SW], f32, tag="n3")
                  nc.vector.tensor_tensor(out=n3[:], in0=n1[:], in1=T1[:], op=A.min)
                  wm1 = wpool.tile([MO, W], f32, tag="mgw")
                  nc.vector.tensor_max(out=wm1[:], in0=v3[:, 5:5 + W], in1=v3[:, 6:6 + W])
                  wmx = wpool.tile([MO, W], f32, tag="wmx")
                  nc.vector.tensor_max(out=wmx[:], in0=wm1[:], in1=v3[:, 7:7 + W])
                  nm1 = wpool.tile([MO, W], f32, tag="mgw")
                  nc.vector.tensor_tensor(out=nm1[:], in0=n3[:, 5:5 + W], in1=n3[:, 6:6 + W], op=A.min)
                  nmn = wpool.tile([MO, W], f32, tag="nmn")
                  nc.vector.tensor_tensor(out=nmn[:], in0=nm1[:], in1=n3[:, 7:7 + W], op=A.min)
                  mgrad = o_new()
                  nc.gpsimd.tensor_sub(out=mgrad[:], in0=wmx[:], in1=nmn[:])
                  dma_out('mgrad', mgrad[:])

                  # ---- structure tensor boxes ----
                  jxx_ps = convB(gxx)
                  jyy_ps = convB(gyy)
                  jyyS = o_new()
                  nc.scalar.copy(out=h3(jyyS[:]), in_=ps3(jyy_ps, NO))
                  tr = o_new()
                  nc.vector.tensor_add(out=h3(tr[:]), in0=ps3(jxx_ps, NO), in1=h3(jyyS[:]))
                  dma_out('gen', tr[:])
                  dd = o_new()
                  nc.vector.tensor_sub(out=h3(dd[:]), in0=ps3(jxx_ps, NO), in1=h3(jyyS[:]))
                  pBj = [(dx, 1 + blk) for dx, _ in P_B]
                  jxy_ps = conv(pBj, 'B', gxy[:ME], bandB, MO, NO)
                  jxyS = o_new()
                  nc.scalar.copy(out=h3(jxyS[:]), in_=ps3(jxy_ps, NO))
                  d2 = o_new()
                  nc.scalar.activation(d2[:], dd[:], AF.Square)
                  jxy2 = o_new()
                  nc.scalar.activation(jxy2[:], jxyS[:], AF.Square)
                  ss = o_new()
                  nc.vector.scalar_tensor_tensor(out=ss[:], in0=jxy2[:], scalar=4.0, in1=d2[:],
                                                 op0=A.mult, op1=A.add)
                  lam = o_new()
                  nc.scalar.activation(lam[:], ss[:], AF.Sqrt, bias=EPS)
                  tre = o_new()
                  nc.vector.tensor_scalar_add(tre[:], tr[:], EPS)
                  rtr = o_new()
                  nc.vector.reciprocal(rtr[:], tre[:])
                  coh = o_new()
                  nc.gpsimd.tensor_mul(out=coh[:], in0=lam[:], in1=rtr[:])
                  dma_out('coherence', coh[:])
                  aniso = o_new()
                  nc.gpsimd.tensor_mul(out=aniso[:], in0=dd[:], in1=rtr[:])
                  dma_out('anisotropy', aniso[:])

                  # ---- residual / checker / stripe energies ----
                  o_copy_out('res_eh', convB(rhsq))
                  o_copy_out('res_ev', convB(rvsq))
                  o_copy_out('chk_e', convB(chksq), eng='s')
                  o_copy_out('str_e', convB(s2), eng='s')

                  # ---- simple O-grid convs ----
                  o_copy_out('lap', convO('lap'))
                  hxx_ps = convO('hxx')
                  abshxx = o_new()
                  nc.scalar.activation(h3(abshxx[:]), ps3(hxx_ps, NO), AF.Abs)
                  o_copy_out('hxx', hxx_ps)
                  hyy_ps = convO('hyy')
                  abshyy = o_new()
                  nc.scalar.activation(h3(abshyy[:]), ps3(hyy_ps, NO), AF.Abs)
                  o_copy_out('hyy', hyy_ps)
                  dsd = o_new()
                  nc.gpsimd.tensor_sub(out=dsd[:], in0=abshxx[:], in1=abshyy[:])
                  dma_out('dsd', dsd[:])
                  o_copy_out('hxy', convO('hxy'))
                  o_copy_out('gdm', convO('gdm'))
                  o_copy_out('gda', convO('gda'), eng='s')
                  o_copy_out('hsh', convO('hsh'), eng='s')
                  o_copy_out('hsv', convO('hsv'), eng='s')
                  o_copy_out('g45', convO('g45'))
                  o_copy_out('g135', convO('g135'))
                  o_copy_out('dctp', convO('dct'), eng='s')

                  # ---- hgh / hgv / mhc ----
                  hgh_ps = convO('hgh')
                  hghS = o_copy_out('hgh', hgh_ps, eng='s')
                  hgv_ps = convO('hgv')
                  hgvS = o_copy_out('hgv', hgv_ps, eng='s')
                  hd = o_new()
                  nc.vector.tensor_sub(out=h3(hd[:]), in0=ps3(hgh_ps, NO), in1=h3(hgvS[:]))
                  had = o_new()
                  nc.scalar.activation(had[:], hd[:], AF.Abs)
                  dma_out('ha_dis', had[:])

                  mhcf_ps = convO('mhcf')
                  bmf = o_new()
                  nc.vector.tensor_sub(out=h3(bmf[:]), in0=h3(bayerO), in1=ps3(mhcf_ps, NO))
                  gbm = o_new()
                  nc.gpsimd.tensor_mul(out=gbm[:], in0=bmf[:], in1=gmask_t[:])
                  mhc = o_new()
                  nc.vector.tensor_add(out=h3(mhc[:]), in0=ps3(mhcf_ps, NO), in1=h3(gbm[:]))
                  dma_out('mhc', mhc[:])
                  mha1 = o_new()
                  nc.vector.scalar_tensor_tensor(out=mha1[:], in0=hghS[:], scalar=-0.5,
                                                 in1=mhc[:], op0=A.mult, op1=A.add)
                  mhc_ha = o_new()
                  nc.vector.scalar_tensor_tensor(out=mhc_ha[:], in0=hgvS[:], scalar=-0.5,
                                                 in1=mha1[:], op0=A.mult, op1=A.add)
                  dma_out('mhc_ha', mhc_ha[:])
                  rres = o_new()
                  nc.gpsimd.tensor_mul(out=rres[:], in0=bmf[:], in1=rbmask_t[:, 0:W])
                  dma_out('rres', rres[:])
                  bres = o_new()
                  nc.gpsimd.tensor_mul(out=bres[:], in0=bmf[:], in1=rbmask_t[:, W:2 * W])
                  dma_out('bres', bres[:])

                  # ---- smooth5 fills (polyphase, 1/den folded into bands) ----
                  strip_pp = strip[:].rearrange("k (c t) -> k c t", t=2)

                  def sm5_conv(groups):
                      # groups: list of (passes, cX); all accumulate into one psum
                      ps = ppo.tile([MO, 1024], f32, tag="po", name="ps")
                      for p in range(2):
                          chain = [(bi, j0, cX) for passes, cX in groups
                                   for (pp_, bi, j0) in passes if pp_ == p]
                          for i, (bi, j0, cX) in enumerate(chain):
                              nc.tensor.matmul(
                                  ps[:, p * 512:p * 512 + NO],
                                  bandO(bi), strip_pp[:, j0:j0 + NO, cX],
                                  start=(i == 0), stop=(i == len(chain) - 1))
                      return ps

                  def ppv(t):
                      # [MO, W] tile viewed phase-major: [MO, 2, 384], col = 2*j + p
                      return t[:].rearrange("p (j t) -> p t j", t=2)

                  gf_ps = sm5_conv([(P_SM5_GF[0], 1), (P_SM5_GF[1], 0)])
                  gfS = o_new()
                  nc.vector.tensor_copy(out=h3(gfS[:]), in_=ps3(gf_ps, NO))
                  gbf_ps = sm5_conv([P_SM5['gbf']])
                  gbfS = o_new()
                  nc.vector.tensor_copy(out=h3(gbfS[:]), in_=ps3(gbf_ps, NO))
                  rf_ps = sm5_conv([P_SM5['rf']])
                  rg = o_new()
                  nc.vector.tensor_sub(out=ppv(rg), in0=ps3(rf_ps, NO), in1=h3(gfS[:]))
                  dma_out('rg', rg[:])
                  bf_ps = sm5_conv([P_SM5['bf']])
                  bg = o_new()
                  nc.vector.tensor_sub(out=ppv(bg), in0=ps3(bf_ps, NO), in1=h3(gfS[:]))
                  dma_out('bg', bg[:])
                  grf_ps = sm5_conv([P_SM5['grf']])
                  gpd = o_new()
                  nc.vector.tensor_sub(out=ppv(gpd), in0=ps3(grf_ps, NO), in1=h3(gbfS[:]))
                  dma_out('gpd', gpd[:])

                  # ---- line variances ----
                  # horizontal: exact f32 5-tap sums (temps ride the epool rotation)
                  sqT = e_roll()
                  nc.scalar.activation(sqT[0:MO, 0:772], T1[:, 4:776], AF.Square)
                  mu = e_roll()
                  nc.vector.tensor_add(out=mu[0:MO, 0:W], in0=T1[:, 4:4 + W],
                                       in1=T1[:, 5:5 + W])
                  mv_ = e_roll()
                  nc.gpsimd.tensor_add(out=mv_[0:MO, 0:W], in0=T1[:, 7:7 + W],
                                       in1=T1[:, 8:8 + W])
                  mw = e_roll()
                  nc.vector.tensor_add(out=mw[0:MO, 0:W], in0=mu[0:MO, 0:W],
                                       in1=mv_[0:MO, 0:W])
                  msum = e_roll()
                  nc.gpsimd.tensor_add(out=msum[0:MO, 0:W], in0=mw[0:MO, 0:W],
                                       in1=T1[:, 6:6 + W])
                  qu = e_roll()
                  nc.vector.tensor_add(out=qu[0:MO, 0:W], in0=sqT[0:MO, 0:W],
                                       in1=sqT[0:MO, 1:1 + W])
                  qv_ = e_roll()
                  nc.gpsimd.tensor_add(out=qv_[0:MO, 0:W], in0=sqT[0:MO, 3:3 + W],
                                       in1=sqT[0:MO, 4:4 + W])
                  qw = e_roll()
                  nc.vector.tensor_add(out=qw[0:MO, 0:W], in0=qu[0:MO, 0:W],
                                       in1=qv_[0:MO, 0:W])
                  qsum = e_roll()
                  nc.gpsimd.tensor_add(out=qsum[0:MO, 0:W], in0=qw[0:MO, 0:W],
                                       in1=sqT[0:MO, 2:2 + W])
                  mh2 = o_new()
                  nc.scalar.activation(mh2[:], msum[0:MO, 0:W], AF.Square, scale=0.2)
                  lvh = o_new()
                  nc.vector.scalar_tensor_tensor(out=lvh[:], in0=qsum[0:MO, 0:W],
                                                 scalar=0.2, in1=mh2[:],
                                                 op0=A.mult, op1=A.subtract)
                  dma_out('lvh', lvh[:])
                  mv_ps = conv_split('avgv5_h', 'avgv5_l', xh_t, xl_t)
                  mv2 = o_new()
                  nc.scalar.activation(h3(mv2[:]), ps3(mv_ps, NO), AF.Square)
                  qv_ps = conv_split('avgv5_h', 'avgv5_l', bsqr, bsql)
                  lvv = o_new()
                  nc.vector.scalar_tensor_tensor(out=h3(lvv[:]), in0=h3(mv2[:]),
                                                 scalar=-1.0, in1=ps3(qv_ps, NO),
                                                 op0=A.mult, op1=A.add)
                  dma_out('lvv', lvv[:])
                  lvd = o_new()
                  nc.gpsimd.tensor_sub(out=lvd[:], in0=lvh[:], in1=lvv[:])
                  dma_out('lvd', lvd[:])
                  alvd = o_new()
                  nc.scalar.activation(alvd[:], lvd[:], AF.Abs)
                  den2 = o_new()
                  nc.gpsimd.tensor_add(out=den2[:], in0=lvh[:], in1=lvv[:])
                  d2e = o_new()
                  nc.vector.tensor_scalar_add(d2e[:], den2[:], EPS)
                  rden2 = o_new()
                  nc.vector.reciprocal(rden2[:], d2e[:])
                  dconf = o_new()
                  nc.gpsimd.tensor_mul(out=dconf[:], in0=alvd[:], in1=rden2[:])
                  dma_out('dconf', dconf[:])

                  # ---- local mean / variance ----
                  lm_ps = convO('box3')
                  lmS = o_copy_out('lmean', lm_ps, eng='s')
                  lm2 = o_new()
                  nc.scalar.activation(lm2[:], lmS[:], AF.Square)
                  lq_ps = convO('box3', rhs=bsqr)
                  lvar = o_new()
                  nc.vector.scalar_tensor_tensor(out=h3(lvar[:]), in0=h3(lm2[:]),
                                                 scalar=-1.0, in1=ps3(lq_ps, NO),
                                                 op0=A.mult, op1=A.add)
                  dma_out('lvar', lvar[:])

    nc.compile()
    return nc


def _get_program(loop=1, timing=False):
    key = (loop, timing)
    if key not in _PROGRAM:
        _PROGRAM[key] = _build_program(loop, timing)
    return _PROGRAM[key]


def _host_constants():
    def kmajor(b):
        n, k, m = b.shape
        return np.ascontiguousarray(np.transpose(b, (1, 0, 2)).reshape(k, n * m))

    consts = {
        "bandsO": kmajor(BANDS_O),
        "bandsE": kmajor(BANDS_E),
        "bandsB": kmajor(BANDS_B),
    }
    # strip row k <-> image row (k - 3): parity (k+1)%2
    # strip col c <-> image col (c - 6): parity (c)%2
    rb = np.zeros((MO, 2 * W), np.float32)
    rb[:, 0:W] = _tile_pattern(_mask_pattern('r'), MO, W)
    rb[:, W:2 * W] = _tile_pattern(_mask_pattern('b'), MO, W)
    consts["rbmask"] = rb
    consts["gmask"] = _tile_pattern(_mask_pattern('g'), MO, W)
    m7 = np.zeros((7, MO, W), np.float32)
    for i, nm in enumerate(['r', 'g', 'b', 'gr', 'gb', 'row', 'col']):
        m7[i] = _tile_pattern(_mask_pattern(nm), MO, W)
    consts["masks7"] = m7
    return consts


def _in_maps(bayer):
    consts = _host_constants()

    def kmajor(bnd):
        n, k, mm = bnd.shape
        return np.ascontiguousarray(np.transpose(bnd, (1, 0, 2)).reshape(k, n * mm))

    padded = np.pad(bayer[:, 0], ((0, 0), (PH, PH), (PW, PW)), mode='reflect')
    in_maps = []
    for c in range(NCORES):
        b, j = divmod(c, CORES_PER_BATCH)
        strip = padded[b, j * RPC: j * RPC + SR, :]
        m = dict(consts)
        m["xs"] = np.ascontiguousarray(strip)
        if j == 0 or j == CORES_PER_BATCH - 1:
            bb = np.stack([_BB_STD,
                           _bb_variant('top') if j == 0 else _BB_STD,
                           _bb_variant('bot') if j == CORES_PER_BATCH - 1 else _BB_STD])
            m["bandsB"] = kmajor(bb)
        in_maps.append(m)
    return in_maps


def kernel(bayer: np.ndarray) -> np.ndarray:
    from concourse.bass_utils import run_bass_kernel_spmd

    bayer = np.asarray(bayer, np.float32)
    assert bayer.shape == (B, 1, H, W), bayer.shape
    nc = _get_program()
    res = run_bass_kernel_spmd(nc, _in_maps(bayer), list(range(NCORES)))
    out = np.zeros((B, 52, H, W), np.float32)
    for c in range(NCORES):
        b, j = divmod(c, CORES_PER_BATCH)
        out[b, :, j * RPC:(j + 1) * RPC, :] = res.results[c]["out"]
    return out



# revision 18
# speedup vs baseline: 2.0772x; 1.5249x over previous
"""Trainium2 Bass kernel for nn_BayerFeatureExtractor.

Computes 52 feature channels from a [2,1,768,768] bayer image, data-parallel
over 8 NeuronCores (each core: one batch image x 192 rows, 2 row-blocks).

Strategy:
  - Host reflect-pads each batch image by (3 rows, 6 cols); each core gets a
    [198, 780] fp32r strip (rows on SBUF partitions).
  - All convolutions run on the TensorEngine as banded matmuls (fp32r, full
    rate): contraction over input rows with a banded weight matrix encoding
    the kernel's row profile; one PSUM-accumulated pass per nonzero kernel
    column, with the moving operand shifted along the free (column) dim.
  - Intermediates that feed 3x3 box filters are computed on an "E" grid
    extended by 1 row/col so the second conv stage needs no partition-offset
    reads (compute engines require partition start 0); reflect behavior of
    intermediates at image borders is exact because every relevant kernel
    profile is symmetric (or enters squared).
  - Pointwise math spread across Vector (DVE), Scalar (ACT), GPSIMD engines.
  - Morphological gradient via 3 row-shifted DMA copies + max/min chains.
"""
import sys
import math

sys.path.insert(0, '/opt/trn_rl_repo')

import numpy as np

EPS = 1e-6

H = 768
W = 768
B = 2
NCORES = 8
CORES_PER_BATCH = 4
RPC = H // CORES_PER_BATCH          # 192 output rows per core
NBLK = 2
BR = RPC // NBLK                    # 96 output rows per block
PH = 3                              # host row padding
PW = 6                              # host col padding
SR = RPC + 2 * PH                   # 198 strip rows
SW = W + 2 * PW                     # 780 strip cols
KIN = BR + 2 * PH                   # 102 contraction rows per block
ME, MO = 98, 96                     # E-grid / O-grid matmul M
NE, NO = 386, 384                   # matmul half widths
EW = 772                            # E tile width (covers out cols -2..769)


# ---------------------------------------------------------------- kernels ---
def _npk(a, s=1.0):
    return np.asarray(a, dtype=np.float32) * np.float32(s)


def _gabor(theta, sigma=1.1, lambd=3.0, gamma=0.65):
    c = np.arange(-2, 3, dtype=np.float32)
    yy, xx = np.meshgrid(c, c, indexing='ij')
    xt = xx * math.cos(theta) + yy * math.sin(theta)
    yt = -xx * math.sin(theta) + yy * math.cos(theta)
    k = np.exp(-(xt ** 2 + gamma ** 2 * yt ** 2) / (2.0 * sigma ** 2)) * np.cos(
        2.0 * math.pi * xt / lambd)
    k = k - k.mean()
    return (k / max(np.abs(k).sum(), 1e-6)).astype(np.float32)


def _dct_like(u=2, v=2, size=5):
    c = np.arange(size, dtype=np.float32)
    yy, xx = np.meshgrid(c, c, indexing='ij')
    k = np.cos(math.pi * (2 * xx + 1) * u / (2 * size)) * np.cos(
        math.pi * (2 * yy + 1) * v / (2 * size))
    k = k - k.mean()
    return (k / max(np.abs(k).sum(), 1e-6)).astype(np.float32)


K_LAP = _npk([[0, 1, 0], [1, -4, 1], [0, 1, 0]])
K_HXX = _npk([[1, -2, 1]])                     # row kernel
K_HYY = _npk([[1], [-2], [1]])                 # col kernel
K_HXY = _npk([[1, 0, -1], [0, 0, 0], [-1, 0, 1]], 0.25)
K_GX = _npk([[-1, 0, 1], [-2, 0, 2], [-1, 0, 1]], 0.125)
K_GY = _npk([[-1, -2, -1], [0, 0, 0], [1, 2, 1]], 0.125)
K_GDM = _npk([[-2, -1, 0], [-1, 0, 1], [0, 1, 2]], 0.125)
K_GDA = _npk([[0, 1, 2], [-1, 0, 1], [-2, -1, 0]], 0.125)
K_CHK = _npk([[1, -1, 1], [-1, 1, -1], [1, -1, 1]], 1.0 / 9.0)

HGH_V = np.array([-0.25, 0.5, 0.5, 0.5, -0.25], np.float32)
K_HSH = _npk([[-0.5, 0.0, 1.0, 0.0, -0.5]])
K_HSV = _npk([[-0.5], [0.0], [1.0], [0.0], [-0.5]])
K_HGH = HGH_V.reshape(1, 5)
K_HGV = HGH_V.reshape(5, 1)
K_MHC = _npk([[0, 0, -1, 0, 0], [0, 0, 2, 0, 0], [-1, 2, 4, 2, -1],
              [0, 0, 2, 0, 0], [0, 0, -1, 0, 0]], 0.125)
K_STX = _npk([[0.25, -1.0, 1.5, -1.0, 0.25]])
K_STY = K_STX.reshape(5, 1).copy()
K_G45 = _gabor(math.pi / 4.0)
K_G135 = _gabor(3.0 * math.pi / 4.0)
K_DCT = _dct_like()
K_SMOOTH5 = (_npk([[1, 2, 3, 2, 1], [2, 4, 6, 4, 2], [3, 6, 9, 6, 3],
                   [2, 4, 6, 4, 2], [1, 2, 3, 2, 1]]) / np.float32(81.0))
K_RESH = (np.eye(1, 5, 2, dtype=np.float32) - K_HGH)      # delta - hgh (1x5)
K_RESV = (np.eye(5, 1, -2, dtype=np.float32) - K_HGV)     # delta - hgv (5x1)
K_BOX3 = np.full((3, 3), 1.0 / 9.0, np.float32)
K_AVGH5 = np.full((1, 5), 0.2, np.float32)
K_AVGV5 = np.full((5, 1), 0.2, np.float32)


def _mask_pattern(name):
    # value at (row parity, col parity), gbrg pattern
    m = np.zeros((2, 2), np.float32)
    if name == 'r':
        m[1, 0] = 1.0
    elif name == 'b':
        m[0, 1] = 1.0
    elif name == 'gr':
        m[1, 1] = 1.0
    elif name == 'gb':
        m[0, 0] = 1.0
    elif name == 'g':
        m[0, 0] = 1.0; m[1, 1] = 1.0
    elif name == 'row':
        m[1, :] = 1.0
    elif name == 'col':
        m[:, 1] = 1.0
    return m


def _den_pattern(name):
    # conv(mask, SMOOTH5) is exactly 2x2-periodic (reflect == parity ext.)
    pat = _mask_pattern(name)
    g = np.zeros((16, 16), np.float32)
    for r in range(16):
        for c in range(16):
            g[r, c] = pat[r % 2, c % 2]
    out = np.zeros((2, 2), np.float32)
    for r in (6, 7):
        for c in (6, 7):
            acc = np.float32(0.0)
            for dy in range(5):
                for dx in range(5):
                    acc += K_SMOOTH5[dy, dx] * g[r + dy - 2, c + dx - 2]
            out[r % 2, c % 2] = acc
    return np.maximum(out, EPS)


def _tile_pattern(pat, rows, cols, row_shift=0, col_shift=0):
    out = np.zeros((rows, cols), np.float32)
    for rp in range(2):
        for cp in range(2):
            out[rp::2, cp::2] = pat[(rp + row_shift) % 2, (cp + col_shift) % 2]
    return out


# ------------------------------------------------------------ band builder ---
class Bands:
    """Dedup banded lhsT matrices per grid ('O' out rows, 'E' extended, 'B' box)."""

    def __init__(self):
        self.items = {'O': [], 'E': [], 'B': []}
        self.index = {}

    def get(self, grid, prof):
        key = (grid, tuple(np.round(np.asarray(prof, np.float64), 10)))
        if key in self.index:
            return self.index[key]
        prof = np.asarray(prof, np.float32)
        kh = len(prof)
        off = kh // 2
        if grid == 'O':
            m = np.zeros((KIN, MO), np.float32)
            for mm in range(MO):
                base = mm + PH - off
                for t in range(kh):
                    m[base + t, mm] = prof[t]
        elif grid == 'E':
            m = np.zeros((KIN, ME), np.float32)
            for ii in range(ME):
                base = ii + PH - 1 - off
                for t in range(kh):
                    m[base + t, ii] = prof[t]
        else:  # 'B': 3-row box applied to E tiles
            m = np.zeros((ME, MO), np.float32)
            for mm in range(MO):
                for t in range(kh):
                    m[mm + t, mm] = prof[t]
        idx = len(self.items[grid])
        self.items[grid].append(m)
        self.index[key] = idx
        return idx

    def passes(self, grid, K):
        K = np.atleast_2d(np.asarray(K, np.float32))
        kw = K.shape[1]
        out = []
        for dxi in range(kw):
            col = K[:, dxi]
            if np.any(col != 0.0):
                out.append((dxi - kw // 2, self.get(grid, col)))
        return out


_BANDS = Bands()

P_E = {
    'gx': _BANDS.passes('E', K_GX),
    'gy': _BANDS.passes('E', K_GY),
    'chk': _BANDS.passes('E', K_CHK),
    'stx': _BANDS.passes('E', K_STX),
    'sty': _BANDS.passes('E', K_STY),
    'resh': _BANDS.passes('E', K_RESH),
    'resv': _BANDS.passes('E', K_RESV),
}
P_O = {
    'lap': _BANDS.passes('O', K_LAP),
    'hxx': _BANDS.passes('O', K_HXX),
    'hyy': _BANDS.passes('O', K_HYY),
    'hxy': _BANDS.passes('O', K_HXY),
    'gdm': _BANDS.passes('O', K_GDM),
    'gda': _BANDS.passes('O', K_GDA),
    'hsh': _BANDS.passes('O', K_HSH),
    'hsv': _BANDS.passes('O', K_HSV),
    'hgh': _BANDS.passes('O', K_HGH),
    'hgv': _BANDS.passes('O', K_HGV),
    'mhcf': _BANDS.passes('O', K_MHC),
    'g45': _BANDS.passes('O', K_G45),
    'g135': _BANDS.passes('O', K_G135),
    'dct': _BANDS.passes('O', K_DCT),
    'sm5': _BANDS.passes('O', K_SMOOTH5),
    'avgh5': _BANDS.passes('O', K_AVGH5),
    'avgv5': _BANDS.passes('O', K_AVGV5),
    'box3': _BANDS.passes('O', K_BOX3),
}
P_B = _BANDS.passes('B', K_BOX3)


BANDS_E = np.stack(_BANDS.items['E'])          # [nE, 102, 98]
def _trunc10(a):
    b = np.asarray(a, np.float32).copy()
    v = b.view(np.uint32)
    v &= np.uint32(0xFFFFE000)
    return b


K_AVGH5_H = _trunc10(K_AVGH5)
K_AVGH5_L = K_AVGH5 - K_AVGH5_H
K_AVGV5_H = _trunc10(K_AVGV5)
K_AVGV5_L = K_AVGV5 - K_AVGV5_H
P_O.update({
    'avgv5_h': _BANDS.passes('O', K_AVGV5_H),
    'avgv5_l': _BANDS.passes('O', K_AVGV5_L),
})

# --- polyphase smooth5 bands: row-parity mask and 1/den folded into bands.
# For mask X (row parity rX, col parity cX) and den channel D, out-col phase
# p: passes use only taps dx with (p+dx)%2 == cX; band rows of wrong parity
# are zeroed; band cols scaled by 1/den_D(out-row parity, p).
_SM5_META = {'r': (1, 0), 'b': (0, 1), 'gr': (1, 1), 'gb': (0, 0)}


def _sm5_passes(X, D):
    rX, cX = _SM5_META[X]
    rden = 1.0 / _den_pattern(D)
    out = []
    for p in range(2):
        for dxi in range(5):
            if (p + dxi) % 2 != cX:
                continue
            m = np.zeros((KIN, MO), np.float32)
            for mm in range(MO):
                for t in range(5):
                    k = mm + 1 + t
                    if (k + 1) % 2 == rX:
                        m[k, mm] = K_SMOOTH5[t, dxi] * rden[mm % 2, p]
            key = ('SM5', X, D, p, dxi)
            if key not in _BANDS.index:
                _BANDS.index[key] = len(_BANDS.items['O'])
                _BANDS.items['O'].append(m)
            # rhs strided-view offset (in cX-parity column units)
            j0 = (PW + p + (dxi - 2) - cX) // 2
            out.append((p, _BANDS.index[key], j0))
    return out


P_SM5 = {
    'rf': (_sm5_passes('r', 'r'), 0),
    'bf': (_sm5_passes('b', 'b'), 1),
    'grf': (_sm5_passes('gr', 'gr'), 1),
    'gbf': (_sm5_passes('gb', 'gb'), 0),
}
P_SM5_GF = (_sm5_passes('gr', 'g'), _sm5_passes('gb', 'g'))  # accumulate both

BANDS_O = np.stack(_BANDS.items['O'])          # [nO, 102, 96]
_BB_STD = _BANDS.items['B'][0]
BANDS_B = np.stack([_BB_STD, _BB_STD, _BB_STD])   # [3, 98, 96]: std, top-slot, bot-slot


def _bb_variant(kind):
    m = _BB_STD.copy()
    if kind == 'top':
        m[0, 0] = -m[0, 0]
    else:
        m[ME - 1, MO - 1] = -m[ME - 1, MO - 1]
    return m

CH = {n: i for i, n in enumerate([
    'r', 'g', 'b', 'gr', 'gb', 'rowm', 'colm',
    'lap', 'hxx', 'hyy', 'hxy', 'mgrad', 'gx', 'gy', 'gdm', 'gda', 'gmag',
    'coherence', 'anisotropy', 'hsh', 'hsv', 'hgh', 'hgv', 'ha_dis',
    'res_h', 'res_v', 'res_eh', 'res_ev', 'dgd', 'dsd', 'lvh', 'lvv', 'lvd',
    'dconf', 'rg', 'bg', 'gpd', 'mhc', 'mhc_ha', 'rres', 'bres',
    'stx', 'sty', 'chk', 'g45', 'g135', 'dctp', 'chk_e', 'str_e', 'lmean',
    'lvar', 'gen'])}


# ------------------------------------------------------------- bass program ---
_PROGRAM = {}


def _build_program(loop=1, timing=False):
    import concourse.bacc as bacc
    import concourse.mybir as mybir
    from concourse.tile import TileContext

    f32 = mybir.dt.float32
    f32r = mybir.dt.float32r
    u8 = mybir.dt.uint8
    A = mybir.AluOpType
    AF = mybir.ActivationFunctionType

    nc = bacc.Bacc("TRN2")

    if timing:
        def declare(name, shape, dtype, isOutput):
            return nc.dram_tensor(name, shape, dtype).ap()
        tin = nc.declare_dram_parameter("tin", [1, 4], mybir.dt.float32, isOutput=False)
        tout = nc.declare_dram_parameter("tout", [1, 4], mybir.dt.float32, isOutput=True)
    else:
        def declare(name, shape, dtype, isOutput):
            return nc.declare_dram_parameter(name, shape, dtype, isOutput=isOutput)

    def register_const(value):
        t = nc.alloc_sbuf_tensor(f"constf32-{value}", [128, 1], f32)
        nc.gpsimd.memset(t.ap(), value)
        nc.const_aps.aps[(f32, value)] = t.ap()

    register_const(EPS)
    nc.all_engine_barrier()

    nO, nE, nB = BANDS_O.shape[0], BANDS_E.shape[0], BANDS_B.shape[0]
    xs_ext = declare("xs", [SR, SW], f32r, isOutput=False)
    bo_ext = declare("bandsO", [KIN, nO * MO], f32r, isOutput=False)
    be_ext = declare("bandsE", [KIN, nE * ME], f32r, isOutput=False)
    bb_ext = declare("bandsB", [ME, nB * MO], f32r, isOutput=False)
    rbmask_ext = declare("rbmask", [MO, 2 * W], f32, isOutput=False)
    gmask_ext = declare("gmask", [MO, W], f32, isOutput=False)
    masks7_ext = declare("masks7", [7, MO, W], f32, isOutput=False)
    out_ext = declare("out", [52, RPC, W], f32, isOutput=True)

    with TileContext(nc) as tc:
        with (
            tc.tile_pool(name="const", bufs=1) as cpool,
            tc.tile_pool(name="work", bufs=1) as wpool,
            tc.tile_pool(name="ebuf", bufs=8) as epool,
            tc.tile_pool(name="obuf", bufs=15) as opool,
            tc.tile_pool(name="psE", bufs=2, space="PSUM") as ppe,
            tc.tile_pool(name="psO", bufs=2, space="PSUM") as ppo,
        ):
            # ---- constants -> SBUF (once) ----
            bo_t = cpool.tile([KIN, nO * MO], f32r)
            nc.sync.dma_start(out=bo_t[:], in_=bo_ext[:])
            be_t = cpool.tile([KIN, nE * ME], f32r)
            nc.sync.dma_start(out=be_t[:], in_=be_ext[:])
            bb_t = cpool.tile([ME, nB * MO], f32r)
            nc.sync.dma_start(out=bb_t[:], in_=bb_ext[:])
            rbmask_t = cpool.tile([MO, 2 * W], f32)
            nc.sync.dma_start(out=rbmask_t[:], in_=rbmask_ext[:])
            gmask_t = cpool.tile([MO, W], f32)
            nc.sync.dma_start(out=gmask_t[:], in_=gmask_ext[:])

            def bandO(i):
                return bo_t[:, i * MO:(i + 1) * MO]

            def bandE(i):
                return be_t[:, i * ME:(i + 1) * ME]

            def bandB(i):
                return bb_t[:, i * MO:(i + 1) * MO]

            def h3(ap):
                # [P, 2N] (possibly offset slice) -> [P, 2, N]
                return ap.rearrange("p (b n) -> p b n", b=2)

            if timing:
                nc.sync.dma_start(out=tout[:], in_=tin[:])
            # mask output channels: DRAM -> DRAM
            for ch_i, name in enumerate(['r', 'g', 'b', 'gr', 'gb', 'rowm', 'colm']):
                for blk in range(NBLK):
                    nc.sync.dma_start(out=out_ext[CH[name], blk * BR:(blk + 1) * BR, :],
                                      in_=masks7_ext[ch_i])

            import contextlib
            loop_cm = tc.For_i(0, loop, 1) if loop > 1 else contextlib.nullcontext()
            with loop_cm:
              for blk in range(NBLK):
                  r0 = blk * BR

                  # ---- inputs ----
                  strip = wpool.tile([KIN, SW], f32r, tag="strip", bufs=2)
                  nc.scalar.dma_start(out=strip[:], in_=xs_ext[r0:r0 + KIN, :])
                  stripf = strip[:].bitcast(f32)
                  T0 = wpool.tile([MO, SW], f32, tag="T0", bufs=2)
                  T1 = wpool.tile([MO, SW], f32, tag="T1", bufs=2)
                  T2 = wpool.tile([MO, SW], f32, tag="T2", bufs=2)
                  nc.scalar.dma_start(out=T0[:], in_=xs_ext[r0 + 2:r0 + 2 + MO, :].bitcast(f32))
                  nc.scalar.dma_start(out=T1[:], in_=xs_ext[r0 + 3:r0 + 3 + MO, :].bitcast(f32))
                  nc.scalar.dma_start(out=T2[:], in_=xs_ext[r0 + 4:r0 + 4 + MO, :].bitcast(f32))

                  bayerO = T1[:, PW:PW + W]

                  # ---- derived matmul inputs ----
                  # strip32 load dropped: strip f32r bytes ARE the f32 values
                  bsq32 = wpool.tile([KIN, SW], f32, tag="bsq32")
                  nc.scalar.activation(bsq32[:], stripf, AF.Square)
                  # hi/lo fp32r split of bayer and bayer^2 for full-precision
                  # variance convs at fp32r matmul rate
                  xh_t = wpool.tile([KIN, SW], f32r, tag="xh")
                  nc.scalar.copy(out=xh_t[:], in_=stripf)
                  xl_t = wpool.tile([KIN, SW], f32r, tag="xl")
                  nc.vector.tensor_sub(out=xl_t[:], in0=stripf, in1=xh_t[:].bitcast(f32))
                  bsqr = wpool.tile([KIN, SW], f32r, tag="bsqr")
                  nc.scalar.activation(bsqr[:], stripf, AF.Square)
                  bsql = wpool.tile([KIN, SW], f32r, tag="bsql")
                  nc.vector.tensor_sub(out=bsql[:], in0=bsq32[:], in1=bsqr[:].bitcast(f32))

                  # ---- conv helpers ----
                  def conv(passes, grid, rhs, band_fn, M, NH):
                      pool = ppe if grid == 'E' else ppo
                      ps = pool.tile([M, 1024], f32, tag="pe" if grid == 'E' else "po",
                                     name="ps")
                      shift = {'E': PW - 2, 'O': PW, 'B': 2}[grid]
                      for h in range(2):
                          for i, (dx, bi) in enumerate(passes):
                              nc.tensor.matmul(
                                  ps[:, h * 512:h * 512 + NH],
                                  band_fn(bi),
                                  rhs[:, shift + dx + h * NH: shift + dx + h * NH + NH],
                                  start=(i == 0), stop=(i == len(passes) - 1))
                      return ps

                  def convE(name, rhs=None):
                      return conv(P_E[name], 'E', (rhs if rhs is not None else strip)[:], bandE, ME, NE)

                  def convO(name, rhs=None):
                      return conv(P_O[name], 'O', (rhs if rhs is not None else strip)[:], bandO, MO, NO)

                  def convB(rhs_tile):
                      return conv(P_B, 'B', rhs_tile[:ME], bandB, MO, NO)

                  def conv_split(kh_name, kl_name, xh, xl):
                      ps = ppo.tile([MO, 1024], f32, tag="po", name="ps")
                      for h in range(2):
                          chain = ([(bi, xh, dx) for dx, bi in P_O[kh_name]]
                                   + [(bi, xl, dx) for dx, bi in P_O[kh_name]]
                                   + [(bi, xh, dx) for dx, bi in P_O[kl_name]])
                          for i, (bi, rhs_t, dx) in enumerate(chain):
                              nc.tensor.matmul(
                                  ps[:, h * 512:h * 512 + NO],
                                  bandO(bi),
                                  rhs_t[:, PW + dx + h * NO: PW + dx + h * NO + NO],
                                  start=(i == 0), stop=(i == len(chain) - 1))
                      return ps


                  def ps3(ps, NH):
                      return h3(ps[:, 0:1024])[:, :, 0:NH]

                  def e_named(tag, dtype=f32):
                      return wpool.tile([ME, EW], dtype, tag=tag, name=tag)

                  def e_roll(dtype=f32):
                      return epool.tile([ME, EW], dtype, tag="ebuf", name="eb")

                  def o_new():
                      return opool.tile([MO, W], f32, tag="obuf", name="ob")

                  def dma_out(name, ap):
                      nc.sync.dma_start(out=out_ext[CH[name], r0:r0 + BR, :], in_=ap)

                  def o_copy_out(name, ps, eng='v'):
                      t = o_new()
                      if eng == 'v':
                          nc.vector.tensor_copy(out=h3(t[:]), in_=ps3(ps, NO))
                      else:
                          nc.scalar.copy(out=h3(t[:]), in_=ps3(ps, NO))
                      dma_out(name, t[:])
                      return t

                  def e_win(t):
                      return t[1:97, 2:770]

                  # ---- E-grid stage 1 + products ----
                  # pattern: ONE copy drains each PSUM tile (frees it for the
                  # next conv); all products read the contiguous SBUF copy.
                  gx_ps = convE('gx')
                  gxS = e_roll()
                  nc.scalar.copy(out=h3(gxS[:]), in_=ps3(gx_ps, NE))
                  dma_out('gx', e_win(gxS))
                  gy_ps = convE('gy')
                  gyS = e_named("gyS")
                  nc.vector.tensor_copy(out=h3(gyS[:]), in_=ps3(gy_ps, NE))
                  dma_out('gy', e_win(gyS))
                  gxx = e_named("gxx", f32r)
                  nc.scalar.activation(gxx[:], gxS[:], AF.Square)
                  gyy = e_named("gyy", f32r)
                  nc.scalar.activation(gyy[:], gyS[:], AF.Square)
                  gxy = e_named("gxy", f32r)
                  nc.vector.tensor_mul(out=gxy[:], in0=gxS[:], in1=gyS[:])
                  # virtual cols -1/768 of the gx*gy product have flipped sign
                  # relative to the reflect of the product; fix before the box.
                  nc.vector.tensor_scalar_mul(out=gxy[:, 1:2], in0=gxy[:, 1:2], scalar1=-1.0)
                  nc.vector.tensor_scalar_mul(out=gxy[:, 770:771], in0=gxy[:, 770:771], scalar1=-1.0)
                  absgx = e_roll()
                  nc.scalar.activation(absgx[:], gxS[:], AF.Abs)
                  absgy = e_roll()
                  nc.scalar.activation(absgy[:], gyS[:], AF.Abs)
                  dgd = e_roll()
                  nc.gpsimd.tensor_sub(out=dgd[:], in0=absgx[:], in1=absgy[:])
                  dma_out('dgd', e_win(dgd))
                  g2 = e_roll()
                  nc.gpsimd.tensor_add(out=g2[:], in0=gxx[:].bitcast(f32), in1=gyy[:].bitcast(f32))
                  gmag = e_roll()
                  nc.scalar.activation(gmag[:], g2[:], AF.Sqrt, bias=EPS)
                  dma_out('gmag', e_win(gmag))

                  chk_ps = convE('chk')
                  chkS = e_roll()
                  nc.scalar.copy(out=h3(chkS[:]), in_=ps3(chk_ps, NE))
                  dma_out('chk', e_win(chkS))
                  chksq = e_named("chksq", f32r)
                  nc.scalar.activation(chksq[:], chkS[:], AF.Square)

                  stx_ps = convE('stx')
                  stxS = e_roll()
                  nc.scalar.copy(out=h3(stxS[:]), in_=ps3(stx_ps, NE))
                  dma_out('stx', e_win(stxS))
                  stxsq = e_named("stxsq")
                  nc.scalar.activation(stxsq[:], stxS[:], AF.Square)
                  sty_ps = convE('sty')
                  styS = e_roll()
                  nc.vector.tensor_copy(out=h3(styS[:]), in_=ps3(sty_ps, NE))
                  dma_out('sty', e_win(styS))
                  stysq = e_named("stysq")
                  nc.scalar.activation(stysq[:], styS[:], AF.Square)
                  s2 = e_named("s2", f32r)
                  nc.gpsimd.tensor_add(out=s2[:], in0=stxsq[:], in1=stysq[:])

                  resh_ps = convE('resh')
                  reshS = e_roll()
                  nc.vector.tensor_copy(out=h3(reshS[:]), in_=ps3(resh_ps, NE))
                  dma_out('res_h', e_win(reshS))
                  rhsq = e_named("rhsq", f32r)
                  nc.scalar.activation(rhsq[:], reshS[:], AF.Square)
                  resv_ps = convE('resv')
                  resvS = e_roll()
                  nc.vector.tensor_copy(out=h3(resvS[:]), in_=ps3(resv_ps, NE))
                  dma_out('res_v', e_win(resvS))
                  rvsq = e_named("rvsq", f32r)
                  nc.scalar.activation(rvsq[:], resvS[:], AF.Square)

                  # ---- simple O-grid convs ----
                  o_copy_out('lap', convO('lap'))
                  hxx_ps = convO('hxx')
                  hxxS = o_copy_out('hxx', hxx_ps)
                  abshxx = o_new()
                  nc.scalar.activation(abshxx[:], hxxS[:], AF.Abs)
                  hyy_ps = convO('hyy')
                  hyyS = o_copy_out('hyy', hyy_ps)
                  abshyy = o_new()
                  nc.scalar.activation(abshyy[:], hyyS[:], AF.Abs)
                  dsd = o_new()
                  nc.gpsimd.tensor_sub(out=dsd[:], in0=abshxx[:], in1=abshyy[:])
                  dma_out('dsd', dsd[:])
                  o_copy_out('hxy', convO('hxy'))
                  o_copy_out('gdm', convO('gdm'))
                  o_copy_out('gda', convO('gda'), eng='s')
                  o_copy_out('hsh', convO('hsh'), eng='s')
                  o_copy_out('hsv', convO('hsv'), eng='s')
                  o_copy_out('g45', convO('g45'))
                  o_copy_out('g135', convO('g135'))
                  o_copy_out('dctp', convO('dct'), eng='s')

                  # ---- mgrad (GPSIMD) ----
                  v1 = wpool.tile([MO, SW], f32, tag="mgtmp")
                  nc.vector.tensor_max(out=v1[:], in0=T0[:], in1=T2[:])
                  v3 = wpool.tile([MO, SW], f32, tag="v3")
                  nc.vector.tensor_max(out=v3[:], in0=v1[:], in1=T1[:])
                  n1 = wpool.tile([MO, SW], f32, tag="mgtmp")
                  nc.vector.tensor_tensor(out=n1[:], in0=T0[:], in1=T2[:], op=A.min)
                  n3 = wpool.tile([MO, SW], f32, tag="n3")
                  nc.vector.tensor_tensor(out=n3[:], in0=n1[:], in1=T1[:], op=A.min)
                  wm1 = wpool.tile([MO, W], f32, tag="mgw")
                  nc.vector.tensor_max(out=wm1[:], in0=v3[:, 5:5 + W], in1=v3[:, 6:6 + W])
                  wmx = wpool.tile([MO, W], f32, tag="wmx")
                  nc.vector.tensor_max(out=wmx[:], in0=wm1[:], in1=v3[:, 7:7 + W])
                  nm1 = wpool.tile([MO, W], f32, tag="mgw")
                  nc.vector.tensor_tensor(out=nm1[:], in0=n3[:, 5:5 + W], in1=n3[:, 6:6 + W], op=A.min)
                  nmn = wpool.tile([MO, W], f32, tag="nmn")
                  nc.vector.tensor_tensor(out=nmn[:], in0=nm1[:], in1=n3[:, 7:7 + W], op=A.min)
                  mgrad = o_new()
                  nc.gpsimd.tensor_sub(out=mgrad[:], in0=wmx[:], in1=nmn[:])
                  dma_out('mgrad', mgrad[:])

                  # ---- structure tensor boxes ----
                  jxx_ps = convB(gxx)
                  jyy_ps = convB(gyy)
                  jyyS = o_new()
                  nc.scalar.copy(out=h3(jyyS[:]), in_=ps3(jyy_ps, NO))
                  tr = o_new()
                  nc.vector.tensor_add(out=h3(tr[:]), in0=ps3(jxx_ps, NO), in1=h3(jyyS[:]))
                  dma_out('gen', tr[:])
                  dd = o_new()
                  nc.vector.scalar_tensor_tensor(out=dd[:], in0=jyyS[:], scalar=-2.0,
                                                 in1=tr[:], op0=A.mult, op1=A.add)
                  pBj = [(dx, 1 + blk) for dx, _ in P_B]
                  jxy_ps = conv(pBj, 'B', gxy[:ME], bandB, MO, NO)
                  jxyS = o_new()
                  nc.scalar.copy(out=h3(jxyS[:]), in_=ps3(jxy_ps, NO))
                  d2 = o_new()
                  nc.scalar.activation(d2[:], dd[:], AF.Square)
                  jxy2 = o_new()
                  nc.scalar.activation(jxy2[:], jxyS[:], AF.Square)
                  ss = o_new()
                  nc.vector.scalar_tensor_tensor(out=ss[:], in0=jxy2[:], scalar=4.0, in1=d2[:],
                                                 op0=A.mult, op1=A.add)
                  lam = o_new()
                  nc.scalar.activation(lam[:], ss[:], AF.Sqrt, bias=EPS)
                  tre = o_new()
                  nc.vector.tensor_scalar_add(tre[:], tr[:], EPS)
                  rtr = o_new()
                  nc.vector.reciprocal(rtr[:], tre[:])
                  coh = o_new()
                  nc.gpsimd.tensor_mul(out=coh[:], in0=lam[:], in1=rtr[:])
                  dma_out('coherence', coh[:])
                  aniso = o_new()
                  nc.gpsimd.tensor_mul(out=aniso[:], in0=dd[:], in1=rtr[:])
                  dma_out('anisotropy', aniso[:])

                  # ---- residual / checker / stripe energies ----
                  o_copy_out('res_eh', convB(rhsq))
                  o_copy_out('res_ev', convB(rvsq))
                  o_copy_out('chk_e', convB(chksq), eng='s')
                  o_copy_out('str_e', convB(s2), eng='s')

                  # ---- hgh / hgv / mhc ----
                  hgh_ps = convO('hgh')
                  hghS = o_copy_out('hgh', hgh_ps, eng='s')
                  hgv_ps = convO('hgv')
                  hgvS = o_copy_out('hgv', hgv_ps, eng='s')
                  hd = o_new()
                  nc.vector.tensor_sub(out=hd[:], in0=hghS[:], in1=hgvS[:])
                  had = o_new()
                  nc.scalar.activation(had[:], hd[:], AF.Abs)
                  dma_out('ha_dis', had[:])

                  mhcf_ps = convO('mhcf')
                  bmf = o_new()
                  nc.vector.tensor_sub(out=h3(bmf[:]), in0=h3(bayerO), in1=ps3(mhcf_ps, NO))
                  gbm = o_new()
                  nc.gpsimd.tensor_mul(out=gbm[:], in0=bmf[:], in1=gmask_t[:])
                  mhc = o_new()
                  nc.vector.tensor_add(out=h3(mhc[:]), in0=ps3(mhcf_ps, NO), in1=h3(gbm[:]))
                  dma_out('mhc', mhc[:])
                  mha1 = o_new()
                  nc.vector.scalar_tensor_tensor(out=mha1[:], in0=hghS[:], scalar=-0.5,
                                                 in1=mhc[:], op0=A.mult, op1=A.add)
                  mhc_ha = o_new()
                  nc.vector.scalar_tensor_tensor(out=mhc_ha[:], in0=hgvS[:], scalar=-0.5,
                                                 in1=mha1[:], op0=A.mult, op1=A.add)
                  dma_out('mhc_ha', mhc_ha[:])
                  rres = o_new()
                  nc.gpsimd.tensor_mul(out=rres[:], in0=bmf[:], in1=rbmask_t[:, 0:W])
                  dma_out('rres', rres[:])
                  bres = o_new()
                  nc.gpsimd.tensor_mul(out=bres[:], in0=bmf[:], in1=rbmask_t[:, W:2 * W])
                  dma_out('bres', bres[:])

                  # ---- smooth5 fills (polyphase, 1/den folded into bands) ----
                  strip_pp = strip[:].rearrange("k (c t) -> k c t", t=2)

                  def sm5_conv(groups):
                      # groups: list of (passes, cX); all accumulate into one psum
                      ps = ppo.tile([MO, 1024], f32, tag="po", name="ps")
                      for p in range(2):
                          chain = [(bi, j0, cX) for passes, cX in groups
                                   for (pp_, bi, j0) in passes if pp_ == p]
                          for i, (bi, j0, cX) in enumerate(chain):
                              nc.tensor.matmul(
                                  ps[:, p * 512:p * 512 + NO],
                                  bandO(bi), strip_pp[:, j0:j0 + NO, cX],
                                  start=(i == 0), stop=(i == len(chain) - 1))
                      return ps

                  def ppv(t):
                      # [MO, W] tile viewed phase-major: [MO, 2, 384], col = 2*j + p
                      return t[:].rearrange("p (j t) -> p t j", t=2)

                  gf_ps = sm5_conv([(P_SM5_GF[0], 1), (P_SM5_GF[1], 0)])
                  gfS = o_new()
                  nc.vector.tensor_copy(out=h3(gfS[:]), in_=ps3(gf_ps, NO))
                  gbf_ps = sm5_conv([P_SM5['gbf']])
                  gbfS = o_new()
                  nc.vector.tensor_copy(out=h3(gbfS[:]), in_=ps3(gbf_ps, NO))
                  rf_ps = sm5_conv([P_SM5['rf']])
                  rg = o_new()
                  nc.vector.tensor_sub(out=ppv(rg), in0=ps3(rf_ps, NO), in1=h3(gfS[:]))
                  dma_out('rg', rg[:])
                  bf_ps = sm5_conv([P_SM5['bf']])
                  bg = o_new()
                  nc.vector.tensor_sub(out=ppv(bg), in0=ps3(bf_ps, NO), in1=h3(gfS[:]))
                  dma_out('bg', bg[:])
                  grf_ps = sm5_conv([P_SM5['grf']])
                  gpd = o_new()
                  nc.vector.tensor_sub(out=ppv(gpd), in0=ps3(grf_ps, NO), in1=h3(gbfS[:]))
                  dma_out('gpd', gpd[:])

                  # ---- line variances ----
                  # horizontal: exact f32 5-tap sums (temps ride the epool rotation)
                  sqT = e_roll()
                  nc.scalar.activation(sqT[0:MO, 0:772], T1[:, 4:776], AF.Square)
                  mu = e_roll()
                  nc.vector.tensor_add(out=mu[0:MO, 0:W], in0=T1[:, 4:4 + W],
                                       in1=T1[:, 5:5 + W])
                  mv_ = e_roll()
                  nc.gpsimd.tensor_add(out=mv_[0:MO, 0:W], in0=T1[:, 7:7 + W],
                                       in1=T1[:, 8:8 + W])
                  mw = e_roll()
                  nc.vector.tensor_add(out=mw[0:MO, 0:W], in0=mu[0:MO, 0:W],
                                       in1=mv_[0:MO, 0:W])
                  msum = e_roll()
                  nc.gpsimd.tensor_add(out=msum[0:MO, 0:W], in0=mw[0:MO, 0:W],
                                       in1=T1[:, 6:6 + W])
                  qu = e_roll()
                  nc.vector.tensor_add(out=qu[0:MO, 0:W], in0=sqT[0:MO, 0:W],
                                       in1=sqT[0:MO, 1:1 + W])
                  qv_ = e_roll()
                  nc.gpsimd.tensor_add(out=qv_[0:MO, 0:W], in0=sqT[0:MO, 3:3 + W],
                                       in1=sqT[0:MO, 4:4 + W])
                  qw = e_roll()
                  nc.vector.tensor_add(out=qw[0:MO, 0:W], in0=qu[0:MO, 0:W],
                                       in1=qv_[0:MO, 0:W])
                  qsum = e_roll()
                  nc.gpsimd.tensor_add(out=qsum[0:MO, 0:W], in0=qw[0:MO, 0:W],
                                       in1=sqT[0:MO, 2:2 + W])
                  mh2 = o_new()
                  nc.scalar.activation(mh2[:], msum[0:MO, 0:W], AF.Square, scale=0.2)
                  lvh = o_new()
                  nc.vector.scalar_tensor_tensor(out=lvh[:], in0=qsum[0:MO, 0:W],
                                                 scalar=0.2, in1=mh2[:],
                                                 op0=A.mult, op1=A.subtract)
                  dma_out('lvh', lvh[:])
                  mv_ps = conv_split('avgv5_h', 'avgv5_l', xh_t, xl_t)
                  mv2 = o_new()
                  nc.scalar.activation(h3(mv2[:]), ps3(mv_ps, NO), AF.Square)
                  qv_ps = conv_split('avgv5_h', 'avgv5_l', bsqr, bsql)
                  lvv = o_new()
                  nc.vector.scalar_tensor_tensor(out=h3(lvv[:]), in0=h3(mv2[:]),
                                                 scalar=-1.0, in1=ps3(qv_ps, NO),
                                                 op0=A.mult, op1=A.add)
                  dma_out('lvv', lvv[:])
                  lvd = o_new()
                  nc.gpsimd.tensor_sub(out=lvd[:], in0=lvh[:], in1=lvv[:])
                  dma_out('lvd', lvd[:])
                  alvd = o_new()
                  nc.scalar.activation(alvd[:], lvd[:], AF.Abs)
                  den2 = o_new()
                  nc.gpsimd.tensor_add(out=den2[:], in0=lvh[:], in1=lvv[:])
                  d2e = o_new()
                  nc.vector.tensor_scalar_add(d2e[:], den2[:], EPS)
                  rden2 = o_new()
                  nc.vector.reciprocal(rden2[:], d2e[:])
                  dconf = o_new()
                  nc.gpsimd.tensor_mul(out=dconf[:], in0=alvd[:], in1=rden2[:])
                  dma_out('dconf', dconf[:])

                  # ---- local mean / variance ----
                  lm_ps = convO('box3')
                  lmS = o_copy_out('lmean', lm_ps, eng='s')
                  lm2 = o_new()
                  nc.scalar.activation(lm2[:], lmS[:], AF.Square)
                  lq_ps = convO('box3', rhs=bsqr)
                  lvar = o_new()
                  nc.vector.scalar_tensor_tensor(out=h3(lvar[:]), in0=h3(lm2[:]),
                                                 scalar=-1.0, in1=ps3(lq_ps, NO),
                                                 op0=A.mult, op1=A.add)
                  dma_out('lvar', lvar[:])

    nc.compile()
    return nc


def _get_program(loop=1, timing=False):
    key = (loop, timing)
    if key not in _PROGRAM:
        _PROGRAM[key] = _build_program(loop, timing)
    return _PROGRAM[key]


def _host_constants():
    def kmajor(b):
        n, k, m = b.shape
        return np.ascontiguousarray(np.transpose(b, (1, 0, 2)).reshape(k, n * m))

    consts = {
        "bandsO": kmajor(BANDS_O),
        "bandsE": kmajor(BANDS_E),
        "bandsB": kmajor(BANDS_B),
    }
    # strip row k <-> image row (k - 3): parity (k+1)%2
    # strip col c <-> image col (c - 6): parity (c)%2
    rb = np.zeros((MO, 2 * W), np.float32)
    rb[:, 0:W] = _tile_pattern(_mask_pattern('r'), MO, W)
    rb[:, W:2 * W] = _tile_pattern(_mask_pattern('b'), MO, W)
    consts["rbmask"] = rb
    consts["gmask"] = _tile_pattern(_mask_pattern('g'), MO, W)
    m7 = np.zeros((7, MO, W), np.float32)
    for i, nm in enumerate(['r', 'g', 'b', 'gr', 'gb', 'row', 'col']):
        m7[i] = _tile_pattern(_mask_pattern(nm), MO, W)
    consts["masks7"] = m7
    return consts


def _in_maps(bayer):
    consts = _host_constants()

    def kmajor(bnd):
        n, k, mm = bnd.shape
        return np.ascontiguousarray(np.transpose(bnd, (1, 0, 2)).reshape(k, n * mm))

    padded = np.pad(bayer[:, 0], ((0, 0), (PH, PH), (PW, PW)), mode='reflect')
    in_maps = []
    for c in range(NCORES):
        b, j = divmod(c, CORES_PER_BATCH)
        strip = padded[b, j * RPC: j * RPC + SR, :]
        m = dict(consts)
        m["xs"] = np.ascontiguousarray(strip)
        if j == 0 or j == CORES_PER_BATCH - 1:
            bb = np.stack([_BB_STD,
                           _bb_variant('top') if j == 0 else _BB_STD,
                           _bb_variant('bot') if j == CORES_PER_BATCH - 1 else _BB_STD])
            m["bandsB"] = kmajor(bb)
        in_maps.append(m)
    return in_maps


def kernel(bayer: np.ndarray) -> np.ndarray:
    from concourse.bass_utils import run_bass_kernel_spmd

    bayer = np.asarray(bayer, np.float32)
    assert bayer.shape == (B, 1, H, W), bayer.shape
    nc = _get_program()
    res = run_bass_kernel_spmd(nc, _in_maps(bayer), list(range(NCORES)))
    out = np.zeros((B, 52, H, W), np.float32)
    for c in range(NCORES):
        b, j = divmod(c, CORES_PER_BATCH)
        out[b, :, j * RPC:(j + 1) * RPC, :] = res.results[c]["out"]
    return out



# revision 22
# speedup vs baseline: 2.7881x; 1.3422x over previous
"""Trainium2 Bass kernel for nn_BayerFeatureExtractor.

Computes 52 feature channels from a [2,1,768,768] bayer image, data-parallel
over 8 NeuronCores (each core: one batch image x 192 rows, 2 row-blocks).

Strategy:
  - Host reflect-pads each batch image by (3 rows, 6 cols); each core gets a
    [198, 780] fp32r strip (rows on SBUF partitions).
  - All convolutions run on the TensorEngine as banded matmuls (fp32r, full
    rate): contraction over input rows with a banded weight matrix encoding
    the kernel's row profile; one PSUM-accumulated pass per nonzero kernel
    column, with the moving operand shifted along the free (column) dim.
  - Intermediates that feed 3x3 box filters are computed on an "E" grid
    extended by 1 row/col so the second conv stage needs no partition-offset
    reads (compute engines require partition start 0); reflect behavior of
    intermediates at image borders is exact because every relevant kernel
    profile is symmetric (or enters squared).
  - Pointwise math spread across Vector (DVE), Scalar (ACT), GPSIMD engines.
  - Morphological gradient via 3 row-shifted DMA copies + max/min chains.
"""
import sys
import math

sys.path.insert(0, '/opt/trn_rl_repo')

import numpy as np

EPS = 1e-6

H = 768
W = 768
B = 2
NCORES = 8
CORES_PER_BATCH = 4
RPC = H // CORES_PER_BATCH          # 192 output rows per core
NBLK = 2
BR = RPC // NBLK                    # 96 output rows per block
PH = 3                              # host row padding
PW = 6                              # host col padding
SR = RPC + 2 * PH                   # 198 strip rows
SW = W + 2 * PW                     # 780 strip cols
KIN = BR + 2 * PH                   # 102 contraction rows per block
ME, MO = 98, 96                     # E-grid / O-grid matmul M
NE, NO = 386, 384                   # matmul half widths
EW = 772                            # E tile width (covers out cols -2..769)


# ---------------------------------------------------------------- kernels ---
def _npk(a, s=1.0):
    return np.asarray(a, dtype=np.float32) * np.float32(s)


def _gabor(theta, sigma=1.1, lambd=3.0, gamma=0.65):
    c = np.arange(-2, 3, dtype=np.float32)
    yy, xx = np.meshgrid(c, c, indexing='ij')
    xt = xx * math.cos(theta) + yy * math.sin(theta)
    yt = -xx * math.sin(theta) + yy * math.cos(theta)
    k = np.exp(-(xt ** 2 + gamma ** 2 * yt ** 2) / (2.0 * sigma ** 2)) * np.cos(
        2.0 * math.pi * xt / lambd)
    k = k - k.mean()
    return (k / max(np.abs(k).sum(), 1e-6)).astype(np.float32)


def _dct_like(u=2, v=2, size=5):
    c = np.arange(size, dtype=np.float32)
    yy, xx = np.meshgrid(c, c, indexing='ij')
    k = np.cos(math.pi * (2 * xx + 1) * u / (2 * size)) * np.cos(
        math.pi * (2 * yy + 1) * v / (2 * size))
    k = k - k.mean()
    return (k / max(np.abs(k).sum(), 1e-6)).astype(np.float32)


K_LAP = _npk([[0, 1, 0], [1, -4, 1], [0, 1, 0]])
K_HXX = _npk([[1, -2, 1]])                     # row kernel
K_HYY = _npk([[1], [-2], [1]])                 # col kernel
K_HXY = _npk([[1, 0, -1], [0, 0, 0], [-1, 0, 1]], 0.25)
K_GX = _npk([[-1, 0, 1], [-2, 0, 2], [-1, 0, 1]], 0.125)
K_GY = _npk([[-1, -2, -1], [0, 0, 0], [1, 2, 1]], 0.125)
K_GDM = _npk([[-2, -1, 0], [-1, 0, 1], [0, 1, 2]], 0.125)
K_GDA = _npk([[0, 1, 2], [-1, 0, 1], [-2, -1, 0]], 0.125)
K_CHK = _npk([[1, -1, 1], [-1, 1, -1], [1, -1, 1]], 1.0 / 9.0)

HGH_V = np.array([-0.25, 0.5, 0.5, 0.5, -0.25], np.float32)
K_HSH = _npk([[-0.5, 0.0, 1.0, 0.0, -0.5]])
K_HSV = _npk([[-0.5], [0.0], [1.0], [0.0], [-0.5]])
K_HGH = HGH_V.reshape(1, 5)
K_HGV = HGH_V.reshape(5, 1)
K_MHC = _npk([[0, 0, -1, 0, 0], [0, 0, 2, 0, 0], [-1, 2, 4, 2, -1],
              [0, 0, 2, 0, 0], [0, 0, -1, 0, 0]], 0.125)
K_STX = _npk([[0.25, -1.0, 1.5, -1.0, 0.25]])
K_STY = K_STX.reshape(5, 1).copy()
K_G45 = _gabor(math.pi / 4.0)
K_G135 = _gabor(3.0 * math.pi / 4.0)
K_DCT = _dct_like()
K_SMOOTH5 = (_npk([[1, 2, 3, 2, 1], [2, 4, 6, 4, 2], [3, 6, 9, 6, 3],
                   [2, 4, 6, 4, 2], [1, 2, 3, 2, 1]]) / np.float32(81.0))
K_RESH = (np.eye(1, 5, 2, dtype=np.float32) - K_HGH)      # delta - hgh (1x5)
K_RESV = (np.eye(5, 1, -2, dtype=np.float32) - K_HGV)     # delta - hgv (5x1)
K_BOX3 = np.full((3, 3), 1.0 / 9.0, np.float32)
K_AVGH5 = np.full((1, 5), 0.2, np.float32)
K_AVGV5 = np.full((5, 1), 0.2, np.float32)


def _mask_pattern(name):
    # value at (row parity, col parity), gbrg pattern
    m = np.zeros((2, 2), np.float32)
    if name == 'r':
        m[1, 0] = 1.0
    elif name == 'b':
        m[0, 1] = 1.0
    elif name == 'gr':
        m[1, 1] = 1.0
    elif name == 'gb':
        m[0, 0] = 1.0
    elif name == 'g':
        m[0, 0] = 1.0; m[1, 1] = 1.0
    elif name == 'row':
        m[1, :] = 1.0
    elif name == 'col':
        m[:, 1] = 1.0
    return m


def _den_pattern(name):
    # conv(mask, SMOOTH5) is exactly 2x2-periodic (reflect == parity ext.)
    pat = _mask_pattern(name)
    g = np.zeros((16, 16), np.float32)
    for r in range(16):
        for c in range(16):
            g[r, c] = pat[r % 2, c % 2]
    out = np.zeros((2, 2), np.float32)
    for r in (6, 7):
        for c in (6, 7):
            acc = np.float32(0.0)
            for dy in range(5):
                for dx in range(5):
                    acc += K_SMOOTH5[dy, dx] * g[r + dy - 2, c + dx - 2]
            out[r % 2, c % 2] = acc
    return np.maximum(out, EPS)


def _tile_pattern(pat, rows, cols, row_shift=0, col_shift=0):
    out = np.zeros((rows, cols), np.float32)
    for rp in range(2):
        for cp in range(2):
            out[rp::2, cp::2] = pat[(rp + row_shift) % 2, (cp + col_shift) % 2]
    return out


# ------------------------------------------------------------ band builder ---
class Bands:
    """Dedup banded lhsT matrices per grid ('O' out rows, 'E' extended, 'B' box)."""

    def __init__(self):
        self.items = {'O': [], 'E': [], 'B': []}
        self.index = {}

    def get(self, grid, prof):
        key = (grid, tuple(np.round(np.asarray(prof, np.float64), 10)))
        if key in self.index:
            return self.index[key]
        prof = np.asarray(prof, np.float32)
        kh = len(prof)
        off = kh // 2
        if grid == 'O':
            m = np.zeros((KIN, MO), np.float32)
            for mm in range(MO):
                base = mm + PH - off
                for t in range(kh):
                    m[base + t, mm] = prof[t]
        elif grid == 'E':
            m = np.zeros((KIN, ME), np.float32)
            for ii in range(ME):
                base = ii + PH - 1 - off
                for t in range(kh):
                    m[base + t, ii] = prof[t]
        else:  # 'B': 3-row box applied to E tiles
            m = np.zeros((ME, MO), np.float32)
            for mm in range(MO):
                for t in range(kh):
                    m[mm + t, mm] = prof[t]
        idx = len(self.items[grid])
        self.items[grid].append(m)
        self.index[key] = idx
        return idx

    def passes(self, grid, K):
        K = np.atleast_2d(np.asarray(K, np.float32))
        kw = K.shape[1]
        out = []
        for dxi in range(kw):
            col = K[:, dxi]
            if np.any(col != 0.0):
                out.append((dxi - kw // 2, self.get(grid, col)))
        return out


_BANDS = Bands()

P_E = {
    'gx': _BANDS.passes('E', K_GX),
    'gy': _BANDS.passes('E', K_GY),
    'chk': _BANDS.passes('E', K_CHK),
    'stx': _BANDS.passes('E', K_STX),
    'sty': _BANDS.passes('E', K_STY),
    'resh': _BANDS.passes('E', K_RESH),
    'resv': _BANDS.passes('E', K_RESV),
}
P_O = {
    'lap': _BANDS.passes('O', K_LAP),
    'hxx': _BANDS.passes('O', K_HXX),
    'hyy': _BANDS.passes('O', K_HYY),
    'hxy': _BANDS.passes('O', K_HXY),
    'gdm': _BANDS.passes('O', K_GDM),
    'gda': _BANDS.passes('O', K_GDA),
    'hsh': _BANDS.passes('O', K_HSH),
    'hsv': _BANDS.passes('O', K_HSV),
    'hgh': _BANDS.passes('O', K_HGH),
    'hgv': _BANDS.passes('O', K_HGV),
    'mhcf': _BANDS.passes('O', K_MHC),
    'g45': _BANDS.passes('O', K_G45),
    'g135': _BANDS.passes('O', K_G135),
    'dct': _BANDS.passes('O', K_DCT),
    'sm5': _BANDS.passes('O', K_SMOOTH5),
    'avgh5': _BANDS.passes('O', K_AVGH5),
    'avgv5': _BANDS.passes('O', K_AVGV5),
    'box3': _BANDS.passes('O', K_BOX3),
}
P_B = _BANDS.passes('B', K_BOX3)


BANDS_E = np.stack(_BANDS.items['E'])          # [nE, 102, 98]
def _trunc10(a):
    b = np.asarray(a, np.float32).copy()
    v = b.view(np.uint32)
    v &= np.uint32(0xFFFFE000)
    return b


K_AVGH5_H = _trunc10(K_AVGH5)
K_AVGH5_L = K_AVGH5 - K_AVGH5_H
K_AVGV5_H = _trunc10(K_AVGV5)
K_AVGV5_L = K_AVGV5 - K_AVGV5_H
P_O.update({
    'avgv5_h': _BANDS.passes('O', K_AVGV5_H),
    'avgv5_l': _BANDS.passes('O', K_AVGV5_L),
})

# --- polyphase smooth5 bands: row-parity mask and 1/den folded into bands.
# For mask X (row parity rX, col parity cX) and den channel D, out-col phase
# p: passes use only taps dx with (p+dx)%2 == cX; band rows of wrong parity
# are zeroed; band cols scaled by 1/den_D(out-row parity, p).
_SM5_META = {'r': (1, 0), 'b': (0, 1), 'gr': (1, 1), 'gb': (0, 0)}


def _sm5_passes(X, D):
    rX, cX = _SM5_META[X]
    rden = 1.0 / _den_pattern(D)
    out = []
    for p in range(2):
        for dxi in range(5):
            if (p + dxi) % 2 != cX:
                continue
            m = np.zeros((KIN, MO), np.float32)
            for mm in range(MO):
                for t in range(5):
                    k = mm + 1 + t
                    if (k + 1) % 2 == rX:
                        m[k, mm] = K_SMOOTH5[t, dxi] * rden[mm % 2, p]
            key = ('SM5', X, D, p, dxi)
            if key not in _BANDS.index:
                _BANDS.index[key] = len(_BANDS.items['O'])
                _BANDS.items['O'].append(m)
            # rhs strided-view offset (in cX-parity column units)
            j0 = (PW + p + (dxi - 2) - cX) // 2
            out.append((p, _BANDS.index[key], j0))
    return out


def _sm5_combined(specs):
    """Fold a signed sum of masked smooth5 fills into one pass set.

    specs: [(mask, den, sign)]; passes with equal (p, dxi, cX) merge by
    summing band matrices (same rhs strided view), so the channel's
    subtraction costs nothing downstream.
    """
    acc = {}
    for X, D, sign in specs:
        rX, cX = _SM5_META[X]
        rden = 1.0 / _den_pattern(D)
        for p in range(2):
            for dxi in range(5):
                if (p + dxi) % 2 != cX:
                    continue
                m = np.zeros((KIN, MO), np.float32)
                for mm in range(MO):
                    for t in range(5):
                        k = mm + 1 + t
                        if (k + 1) % 2 == rX:
                            m[k, mm] = K_SMOOTH5[t, dxi] * rden[mm % 2, p]
                j0 = (PW + p + (dxi - 2) - cX) // 2
                key = (p, dxi, cX, j0)
                acc[key] = acc.get(key, 0.0) + np.float32(sign) * m
    out = []
    for (p, dxi, cX, j0), m in sorted(acc.items()):
        idx = len(_BANDS.items['O'])
        _BANDS.items['O'].append(np.asarray(m, np.float32))
        out.append((p, idx, j0, cX))
    return out


P_SM5D = {
    'rg': _sm5_combined([('r', 'r', 1), ('gr', 'g', -1), ('gb', 'g', -1)]),
    'bg': _sm5_combined([('b', 'b', 1), ('gr', 'g', -1), ('gb', 'g', -1)]),
    'gpd': _sm5_combined([('gr', 'gr', 1), ('gb', 'gb', -1)]),
}

BANDS_O = np.stack(_BANDS.items['O'])          # [nO, 102, 96]
_BB_STD = _BANDS.items['B'][0]
BANDS_B = np.stack([_BB_STD, _BB_STD, _BB_STD])   # [3, 98, 96]: std, top-slot, bot-slot


def _bb_variant(kind):
    m = _BB_STD.copy()
    if kind == 'top':
        m[0, 0] = -m[0, 0]
    else:
        m[ME - 1, MO - 1] = -m[ME - 1, MO - 1]
    return m

CH = {n: i for i, n in enumerate([
    'r', 'g', 'b', 'gr', 'gb', 'rowm', 'colm',
    'lap', 'hxx', 'hyy', 'hxy', 'mgrad', 'gx', 'gy', 'gdm', 'gda', 'gmag',
    'coherence', 'anisotropy', 'hsh', 'hsv', 'hgh', 'hgv', 'ha_dis',
    'res_h', 'res_v', 'res_eh', 'res_ev', 'dgd', 'dsd', 'lvh', 'lvv', 'lvd',
    'dconf', 'rg', 'bg', 'gpd', 'mhc', 'mhc_ha', 'rres', 'bres',
    'stx', 'sty', 'chk', 'g45', 'g135', 'dctp', 'chk_e', 'str_e', 'lmean',
    'lvar', 'gen'])}


# ------------------------------------------------------------- bass program ---
_PROGRAM = {}


def _build_program(loop=1, timing=False):
    import concourse.bacc as bacc
    import concourse.mybir as mybir
    from concourse.tile import TileContext

    f32 = mybir.dt.float32
    f32r = mybir.dt.float32r
    u8 = mybir.dt.uint8
    A = mybir.AluOpType
    AF = mybir.ActivationFunctionType

    nc = bacc.Bacc("TRN2")

    if timing:
        def declare(name, shape, dtype, isOutput):
            return nc.dram_tensor(name, shape, dtype).ap()
        tin = nc.declare_dram_parameter("tin", [1, 4], mybir.dt.float32, isOutput=False)
        tout = nc.declare_dram_parameter("tout", [1, 4], mybir.dt.float32, isOutput=True)
    else:
        def declare(name, shape, dtype, isOutput):
            return nc.declare_dram_parameter(name, shape, dtype, isOutput=isOutput)

    def register_const(value):
        t = nc.alloc_sbuf_tensor(f"constf32-{value}", [128, 1], f32)
        nc.gpsimd.memset(t.ap(), value)
        nc.const_aps.aps[(f32, value)] = t.ap()

    register_const(EPS)
    nc.all_engine_barrier()

    nO, nE, nB = BANDS_O.shape[0], BANDS_E.shape[0], BANDS_B.shape[0]
    xs_ext = declare("xs", [SR, SW], f32r, isOutput=False)
    bo_ext = declare("bandsO", [KIN, nO * MO], f32r, isOutput=False)
    be_ext = declare("bandsE", [KIN, nE * ME], f32r, isOutput=False)
    bb_ext = declare("bandsB", [ME, nB * MO], f32r, isOutput=False)
    rbmask_ext = declare("rbmask", [MO, 2 * W], f32, isOutput=False)
    gmask_ext = declare("gmask", [MO, W], f32, isOutput=False)
    masks7_ext = declare("masks7", [7, MO, W], f32, isOutput=False)
    out_ext = declare("out", [52, RPC, W], f32, isOutput=True)

    with TileContext(nc) as tc:
        with (
            tc.tile_pool(name="const", bufs=1) as cpool,
            tc.tile_pool(name="work", bufs=1) as wpool,
            tc.tile_pool(name="ebuf", bufs=8) as epool,
            tc.tile_pool(name="obuf", bufs=15) as opool,
            tc.tile_pool(name="psE", bufs=2, space="PSUM") as ppe,
            tc.tile_pool(name="psO", bufs=2, space="PSUM") as ppo,
        ):
            # ---- constants -> SBUF (once) ----
            bo_t = cpool.tile([KIN, nO * MO], f32r)
            nc.sync.dma_start(out=bo_t[:], in_=bo_ext[:])
            be_t = cpool.tile([KIN, nE * ME], f32r)
            nc.sync.dma_start(out=be_t[:], in_=be_ext[:])
            bb_t = cpool.tile([ME, nB * MO], f32r)
            nc.sync.dma_start(out=bb_t[:], in_=bb_ext[:])
            rbmask_t = cpool.tile([MO, 2 * W], f32)
            nc.sync.dma_start(out=rbmask_t[:], in_=rbmask_ext[:])
            gmask_t = cpool.tile([MO, W], f32)
            nc.sync.dma_start(out=gmask_t[:], in_=gmask_ext[:])

            def bandO(i):
                return bo_t[:, i * MO:(i + 1) * MO]

            def bandE(i):
                return be_t[:, i * ME:(i + 1) * ME]

            def bandB(i):
                return bb_t[:, i * MO:(i + 1) * MO]

            def h3(ap):
                # [P, 2N] (possibly offset slice) -> [P, 2, N]
                return ap.rearrange("p (b n) -> p b n", b=2)

            if timing:
                nc.sync.dma_start(out=tout[:], in_=tin[:])
            # mask output channels: DRAM -> DRAM
            for ch_i, name in enumerate(['r', 'g', 'b', 'gr', 'gb', 'rowm', 'colm']):
                for blk in range(NBLK):
                    nc.sync.dma_start(out=out_ext[CH[name], blk * BR:(blk + 1) * BR, :],
                                      in_=masks7_ext[ch_i])

            import contextlib
            loop_cm = tc.For_i(0, loop, 1) if loop > 1 else contextlib.nullcontext()
            with loop_cm:
              for blk in range(NBLK):
                  r0 = blk * BR

                  # ---- inputs ----
                  strip = wpool.tile([KIN, SW], f32r, tag="strip", bufs=2)
                  nc.scalar.dma_start(out=strip[:], in_=xs_ext[r0:r0 + KIN, :])
                  stripf = strip[:].bitcast(f32)
                  T0 = wpool.tile([MO, SW], f32, tag="T0", bufs=2)
                  T1 = wpool.tile([MO, SW], f32, tag="T1", bufs=2)
                  T2 = wpool.tile([MO, SW], f32, tag="T2", bufs=2)
                  nc.scalar.dma_start(out=T0[:], in_=xs_ext[r0 + 2:r0 + 2 + MO, :].bitcast(f32))
                  nc.scalar.dma_start(out=T1[:], in_=xs_ext[r0 + 3:r0 + 3 + MO, :].bitcast(f32))
                  nc.scalar.dma_start(out=T2[:], in_=xs_ext[r0 + 4:r0 + 4 + MO, :].bitcast(f32))

                  bayerO = T1[:, PW:PW + W]

                  # ---- derived matmul inputs ----
                  # strip32 load dropped: strip f32r bytes ARE the f32 values
                  bsq32 = wpool.tile([KIN, SW], f32, tag="bsq32")
                  nc.scalar.activation(bsq32[:], stripf, AF.Square)
                  # hi/lo fp32r split of bayer and bayer^2 for full-precision
                  # variance convs at fp32r matmul rate
                  xh_t = wpool.tile([KIN, SW], f32r, tag="xh")
                  nc.scalar.copy(out=xh_t[:], in_=stripf)
                  xl_t = wpool.tile([KIN, SW], f32r, tag="xl")
                  nc.vector.tensor_sub(out=xl_t[:], in0=stripf, in1=xh_t[:].bitcast(f32))
                  bsqr = wpool.tile([KIN, SW], f32r, tag="bsqr")
                  nc.scalar.activation(bsqr[:], stripf, AF.Square)
                  bsql = wpool.tile([KIN, SW], f32r, tag="bsql")
                  nc.vector.tensor_sub(out=bsql[:], in0=bsq32[:], in1=bsqr[:].bitcast(f32))

                  # ---- conv helpers ----
                  def conv(passes, grid, rhs, band_fn, M, NH):
                      pool = ppe if grid == 'E' else ppo
                      ps = pool.tile([M, 1024], f32, tag="pe" if grid == 'E' else "po",
                                     name="ps")
                      shift = {'E': PW - 2, 'O': PW, 'B': 2}[grid]
                      for h in range(2):
                          for i, (dx, bi) in enumerate(passes):
                              nc.tensor.matmul(
                                  ps[:, h * 512:h * 512 + NH],
                                  band_fn(bi),
                                  rhs[:, shift + dx + h * NH: shift + dx + h * NH + NH],
                                  start=(i == 0), stop=(i == len(passes) - 1))
                      return ps

                  def convE(name, rhs=None):
                      return conv(P_E[name], 'E', (rhs if rhs is not None else strip)[:], bandE, ME, NE)

                  def convO(name, rhs=None):
                      return conv(P_O[name], 'O', (rhs if rhs is not None else strip)[:], bandO, MO, NO)

                  def convB(rhs_tile):
                      return conv(P_B, 'B', rhs_tile[:ME], bandB, MO, NO)

                  def conv_split(kh_name, kl_name, xh, xl):
                      ps = ppo.tile([MO, 1024], f32, tag="po", name="ps")
                      for h in range(2):
                          chain = ([(bi, xh, dx) for dx, bi in P_O[kh_name]]
                                   + [(bi, xl, dx) for dx, bi in P_O[kh_name]]
                                   + [(bi, xh, dx) for dx, bi in P_O[kl_name]])
                          for i, (bi, rhs_t, dx) in enumerate(chain):
                              nc.tensor.matmul(
                                  ps[:, h * 512:h * 512 + NO],
                                  bandO(bi),
                                  rhs_t[:, PW + dx + h * NO: PW + dx + h * NO + NO],
                                  start=(i == 0), stop=(i == len(chain) - 1))
                      return ps


                  def ps3(ps, NH):
                      return h3(ps[:, 0:1024])[:, :, 0:NH]

                  def e_named(tag, dtype=f32):
                      return wpool.tile([ME, EW], dtype, tag=tag, name=tag)

                  def e_roll(dtype=f32):
                      return epool.tile([ME, EW], dtype, tag="ebuf", name="eb")

                  def o_new():
                      return opool.tile([MO, W], f32, tag="obuf", name="ob")

                  def dma_out(name, ap):
                      nc.sync.dma_start(out=out_ext[CH[name], r0:r0 + BR, :], in_=ap)

                  def o_copy_out(name, ps, eng='v'):
                      t = o_new()
                      if eng == 'v':
                          nc.vector.tensor_copy(out=h3(t[:]), in_=ps3(ps, NO))
                      else:
                          nc.scalar.copy(out=h3(t[:]), in_=ps3(ps, NO))
                      dma_out(name, t[:])
                      return t

                  def e_win(t):
                      return t[1:97, 2:770]

                  # ---- E-grid stage 1 + products ----
                  # pattern: ONE copy drains each PSUM tile (frees it for the
                  # next conv); all products read the contiguous SBUF copy.
                  gx_ps = convE('gx')
                  gxS = e_roll()
                  nc.scalar.copy(out=h3(gxS[:]), in_=ps3(gx_ps, NE))
                  dma_out('gx', e_win(gxS))
                  gy_ps = convE('gy')
                  gyS = e_named("gyS")
                  nc.vector.tensor_copy(out=h3(gyS[:]), in_=ps3(gy_ps, NE))
                  dma_out('gy', e_win(gyS))
                  gxx = e_named("gxx", f32r)
                  nc.scalar.activation(gxx[:], gxS[:], AF.Square)
                  gyy = e_named("gyy", f32r)
                  nc.scalar.activation(gyy[:], gyS[:], AF.Square)
                  gxy = e_named("gxy", f32r)
                  nc.vector.tensor_mul(out=gxy[:], in0=gxS[:], in1=gyS[:])
                  # virtual cols -1/768 of the gx*gy product have flipped sign
                  # relative to the reflect of the product; fix before the box.
                  nc.vector.tensor_scalar_mul(out=gxy[:, 1:2], in0=gxy[:, 1:2], scalar1=-1.0)
                  nc.vector.tensor_scalar_mul(out=gxy[:, 770:771], in0=gxy[:, 770:771], scalar1=-1.0)
                  absgx = e_roll()
                  nc.scalar.activation(absgx[:], gxS[:], AF.Abs)
                  absgy = e_roll()
                  nc.scalar.activation(absgy[:], gyS[:], AF.Abs)
                  dgd = e_roll()
                  nc.gpsimd.tensor_sub(out=dgd[:], in0=absgx[:], in1=absgy[:])
                  dma_out('dgd', e_win(dgd))
                  g2 = e_roll()
                  nc.gpsimd.tensor_add(out=g2[:], in0=gxx[:].bitcast(f32), in1=gyy[:].bitcast(f32))
                  gmag = e_roll()
                  nc.scalar.activation(gmag[:], g2[:], AF.Sqrt, bias=EPS)
                  dma_out('gmag', e_win(gmag))

                  chk_ps = convE('chk')
                  chkS = e_roll()
                  nc.scalar.copy(out=h3(chkS[:]), in_=ps3(chk_ps, NE))
                  dma_out('chk', e_win(chkS))
                  chksq = e_named("chksq", f32r)
                  nc.scalar.activation(chksq[:], chkS[:], AF.Square)

                  stx_ps = convE('stx')
                  stxS = e_roll()
                  nc.scalar.copy(out=h3(stxS[:]), in_=ps3(stx_ps, NE))
                  dma_out('stx', e_win(stxS))
                  stxsq = e_named("stxsq")
                  nc.scalar.activation(stxsq[:], stxS[:], AF.Square)
                  sty_ps = convE('sty')
                  styS = e_roll()
                  nc.vector.tensor_copy(out=h3(styS[:]), in_=ps3(sty_ps, NE))
                  dma_out('sty', e_win(styS))
                  stysq = e_named("stysq")
                  nc.scalar.activation(stysq[:], styS[:], AF.Square)
                  s2 = e_named("s2", f32r)
                  nc.gpsimd.tensor_add(out=s2[:], in0=stxsq[:], in1=stysq[:])

                  resh_ps = convE('resh')
                  reshS = e_roll()
                  nc.vector.tensor_copy(out=h3(reshS[:]), in_=ps3(resh_ps, NE))
                  dma_out('res_h', e_win(reshS))
                  rhsq = e_named("rhsq", f32r)
                  nc.scalar.activation(rhsq[:], reshS[:], AF.Square)
                  resv_ps = convE('resv')
                  resvS = e_roll()
                  nc.vector.tensor_copy(out=h3(resvS[:]), in_=ps3(resv_ps, NE))
                  dma_out('res_v', e_win(resvS))
                  rvsq = e_named("rvsq", f32r)
                  nc.scalar.activation(rvsq[:], resvS[:], AF.Square)

                  # ---- simple O-grid convs ----
                  hxx_ps = convO('hxx')
                  hxxS = o_copy_out('hxx', hxx_ps)
                  abshxx = o_new()
                  nc.scalar.activation(abshxx[:], hxxS[:], AF.Abs)
                  hyy_ps = convO('hyy')
                  hyyS = o_copy_out('hyy', hyy_ps)
                  abshyy = o_new()
                  nc.scalar.activation(abshyy[:], hyyS[:], AF.Abs)
                  dsd = o_new()
                  nc.gpsimd.tensor_sub(out=dsd[:], in0=abshxx[:], in1=abshyy[:])
                  dma_out('dsd', dsd[:])
                  lap = o_new()
                  nc.gpsimd.tensor_add(out=lap[:], in0=hxxS[:], in1=hyyS[:])
                  dma_out('lap', lap[:])
                  o_copy_out('hxy', convO('hxy'))
                  o_copy_out('gdm', convO('gdm'))
                  o_copy_out('gda', convO('gda'), eng='s')
                  o_copy_out('hsh', convO('hsh'), eng='s')
                  o_copy_out('hsv', convO('hsv'), eng='s')
                  o_copy_out('g45', convO('g45'))
                  o_copy_out('g135', convO('g135'))
                  o_copy_out('dctp', convO('dct'), eng='s')

                  # ---- mgrad (GPSIMD) ----
                  v1 = wpool.tile([MO, SW], f32, tag="mgtmp")
                  nc.vector.tensor_max(out=v1[:], in0=T0[:], in1=T2[:])
                  v3 = wpool.tile([MO, SW], f32, tag="v3")
                  nc.vector.tensor_max(out=v3[:], in0=v1[:], in1=T1[:])
                  n1 = wpool.tile([MO, SW], f32, tag="mgtmp")
                  nc.vector.tensor_tensor(out=n1[:], in0=T0[:], in1=T2[:], op=A.min)
                  n3 = wpool.tile([MO, SW], f32, tag="n3")
                  nc.vector.tensor_tensor(out=n3[:], in0=n1[:], in1=T1[:], op=A.min)
                  wm1 = wpool.tile([MO, W], f32, tag="mgw")
                  nc.vector.tensor_max(out=wm1[:], in0=v3[:, 5:5 + W], in1=v3[:, 6:6 + W])
                  wmx = wpool.tile([MO, W], f32, tag="wmx")
                  nc.vector.tensor_max(out=wmx[:], in0=wm1[:], in1=v3[:, 7:7 + W])
                  nm1 = wpool.tile([MO, W], f32, tag="mgw")
                  nc.vector.tensor_tensor(out=nm1[:], in0=n3[:, 5:5 + W], in1=n3[:, 6:6 + W], op=A.min)
                  nmn = wpool.tile([MO, W], f32, tag="nmn")
                  nc.vector.tensor_tensor(out=nmn[:], in0=nm1[:], in1=n3[:, 7:7 + W], op=A.min)
                  mgrad = o_new()
                  nc.vector.tensor_sub(out=mgrad[:], in0=wmx[:], in1=nmn[:])
                  dma_out('mgrad', mgrad[:])

                  # ---- structure tensor boxes ----
                  jxx_ps = convB(gxx)
                  jyy_ps = convB(gyy)
                  jyyS = o_new()
                  nc.scalar.copy(out=h3(jyyS[:]), in_=ps3(jyy_ps, NO))
                  tr = o_new()
                  nc.vector.tensor_add(out=h3(tr[:]), in0=ps3(jxx_ps, NO), in1=h3(jyyS[:]))
                  dma_out('gen', tr[:])
                  dd = o_new()
                  nc.vector.scalar_tensor_tensor(out=dd[:], in0=jyyS[:], scalar=-2.0,
                                                 in1=tr[:], op0=A.mult, op1=A.add)
                  pBj = [(dx, 1 + blk) for dx, _ in P_B]
                  jxy_ps = conv(pBj, 'B', gxy[:ME], bandB, MO, NO)
                  jxyS = o_new()
                  nc.scalar.copy(out=h3(jxyS[:]), in_=ps3(jxy_ps, NO))
                  d2 = o_new()
                  nc.scalar.activation(d2[:], dd[:], AF.Square)
                  jxy2 = o_new()
                  nc.scalar.activation(jxy2[:], jxyS[:], AF.Square)
                  ss = o_new()
                  nc.vector.scalar_tensor_tensor(out=ss[:], in0=jxy2[:], scalar=4.0, in1=d2[:],
                                                 op0=A.mult, op1=A.add)
                  lam = o_new()
                  nc.scalar.activation(lam[:], ss[:], AF.Sqrt, bias=EPS)
                  tre = o_new()
                  nc.vector.tensor_scalar_add(tre[:], tr[:], EPS)
                  rtr = o_new()
                  nc.vector.reciprocal(rtr[:], tre[:])
                  coh = o_new()
                  nc.gpsimd.tensor_mul(out=coh[:], in0=lam[:], in1=rtr[:])
                  dma_out('coherence', coh[:])
                  aniso = o_new()
                  nc.gpsimd.tensor_mul(out=aniso[:], in0=dd[:], in1=rtr[:])
                  dma_out('anisotropy', aniso[:])

                  # ---- residual / checker / stripe energies ----
                  o_copy_out('res_eh', convB(rhsq))
                  o_copy_out('res_ev', convB(rvsq))
                  o_copy_out('chk_e', convB(chksq), eng='s')
                  o_copy_out('str_e', convB(s2), eng='s')

                  # ---- hgh / hgv / mhc ----
                  hgh_ps = convO('hgh')
                  hghS = o_copy_out('hgh', hgh_ps, eng='s')
                  hgv_ps = convO('hgv')
                  hgvS = o_copy_out('hgv', hgv_ps, eng='s')
                  hd = o_new()
                  nc.vector.tensor_sub(out=hd[:], in0=hghS[:], in1=hgvS[:])
                  had = o_new()
                  nc.scalar.activation(had[:], hd[:], AF.Abs)
                  dma_out('ha_dis', had[:])

                  mhcf_ps = convO('mhcf')
                  bmf = o_new()
                  nc.vector.tensor_sub(out=h3(bmf[:]), in0=h3(bayerO), in1=ps3(mhcf_ps, NO))
                  gbm = o_new()
                  nc.gpsimd.tensor_mul(out=gbm[:], in0=bmf[:], in1=gmask_t[:])
                  mhc = o_new()
                  nc.vector.tensor_add(out=h3(mhc[:]), in0=ps3(mhcf_ps, NO), in1=h3(gbm[:]))
                  dma_out('mhc', mhc[:])
                  mha1 = o_new()
                  nc.vector.scalar_tensor_tensor(out=mha1[:], in0=hghS[:], scalar=-0.5,
                                                 in1=mhc[:], op0=A.mult, op1=A.add)
                  mhc_ha = o_new()
                  nc.vector.scalar_tensor_tensor(out=mhc_ha[:], in0=hgvS[:], scalar=-0.5,
                                                 in1=mha1[:], op0=A.mult, op1=A.add)
                  dma_out('mhc_ha', mhc_ha[:])
                  rres = o_new()
                  nc.gpsimd.tensor_mul(out=rres[:], in0=bmf[:], in1=rbmask_t[:, 0:W])
                  dma_out('rres', rres[:])
                  bres = o_new()
                  nc.gpsimd.tensor_mul(out=bres[:], in0=bmf[:], in1=rbmask_t[:, W:2 * W])
                  dma_out('bres', bres[:])

                  # ---- smooth5 fills (polyphase, 1/den folded into bands) ----
                  strip_pp = strip[:].rearrange("k (c t) -> k c t", t=2)

                  def sm5_conv(passes):
                      ps = ppo.tile([MO, 1024], f32, tag="po", name="ps")
                      for p in range(2):
                          chain = [x for x in passes if x[0] == p]
                          for i, (pp_, bi, j0, cX) in enumerate(chain):
                              nc.tensor.matmul(
                                  ps[:, p * 512:p * 512 + NO],
                                  bandO(bi), strip_pp[:, j0:j0 + NO, cX],
                                  start=(i == 0), stop=(i == len(chain) - 1))
                      return ps

                  def ppv(t):
                      # [MO, W] tile viewed phase-major: [MO, 2, 384], col = 2*j + p
                      return t[:].rearrange("p (j t) -> p t j", t=2)

                  rg_ps = sm5_conv(P_SM5D['rg'])
                  rg = o_new()
                  nc.vector.tensor_copy(out=ppv(rg), in_=ps3(rg_ps, NO))
                  dma_out('rg', rg[:])
                  bg_ps = sm5_conv(P_SM5D['bg'])
                  bg = o_new()
                  nc.scalar.copy(out=ppv(bg), in_=ps3(bg_ps, NO))
                  dma_out('bg', bg[:])
                  gpd_ps = sm5_conv(P_SM5D['gpd'])
                  gpd = o_new()
                  nc.vector.tensor_copy(out=ppv(gpd), in_=ps3(gpd_ps, NO))
                  dma_out('gpd', gpd[:])

                  # ---- line variances ----
                  # horizontal: exact f32 5-tap sums (temps ride the epool rotation)
                  sqT = e_roll()
                  nc.scalar.activation(sqT[0:MO, 0:772], T1[:, 4:776], AF.Square)
                  mu = e_roll()
                  nc.vector.tensor_add(out=mu[0:MO, 0:W], in0=T1[:, 4:4 + W],
                                       in1=T1[:, 5:5 + W])
                  mv_ = e_roll()
                  nc.gpsimd.tensor_add(out=mv_[0:MO, 0:W], in0=T1[:, 7:7 + W],
                                       in1=T1[:, 8:8 + W])
                  mw = e_roll()
                  nc.vector.tensor_add(out=mw[0:MO, 0:W], in0=mu[0:MO, 0:W],
                                       in1=mv_[0:MO, 0:W])
                  msum = e_roll()
                  nc.gpsimd.tensor_add(out=msum[0:MO, 0:W], in0=mw[0:MO, 0:W],
                                       in1=T1[:, 6:6 + W])
                  qu = e_roll()
                  nc.vector.tensor_add(out=qu[0:MO, 0:W], in0=sqT[0:MO, 0:W],
                                       in1=sqT[0:MO, 1:1 + W])
                  qv_ = e_roll()
                  nc.gpsimd.tensor_add(out=qv_[0:MO, 0:W], in0=sqT[0:MO, 3:3 + W],
                                       in1=sqT[0:MO, 4:4 + W])
                  qw = e_roll()
                  nc.vector.tensor_add(out=qw[0:MO, 0:W], in0=qu[0:MO, 0:W],
                                       in1=qv_[0:MO, 0:W])
                  qsum = e_roll()
                  nc.gpsimd.tensor_add(out=qsum[0:MO, 0:W], in0=qw[0:MO, 0:W],
                                       in1=sqT[0:MO, 2:2 + W])
                  mh2 = o_new()
                  nc.scalar.activation(mh2[:], msum[0:MO, 0:W], AF.Square, scale=0.2)
                  lvh = o_new()
                  nc.vector.scalar_tensor_tensor(out=lvh[:], in0=qsum[0:MO, 0:W],
                                                 scalar=0.2, in1=mh2[:],
                                                 op0=A.mult, op1=A.subtract)
                  dma_out('lvh', lvh[:])
                  mv_ps = conv_split('avgv5_h', 'avgv5_l', xh_t, xl_t)
                  mv2 = o_new()
                  nc.scalar.activation(h3(mv2[:]), ps3(mv_ps, NO), AF.Square)
                  qv_ps = conv_split('avgv5_h', 'avgv5_l', bsqr, bsql)
                  lvv = o_new()
                  nc.vector.scalar_tensor_tensor(out=h3(lvv[:]), in0=h3(mv2[:]),
                                                 scalar=-1.0, in1=ps3(qv_ps, NO),
                                                 op0=A.mult, op1=A.add)
                  dma_out('lvv', lvv[:])
                  lvd = o_new()
                  nc.gpsimd.tensor_sub(out=lvd[:], in0=lvh[:], in1=lvv[:])
                  dma_out('lvd', lvd[:])
                  alvd = o_new()
                  nc.scalar.activation(alvd[:], lvd[:], AF.Abs)
                  den2 = o_new()
                  nc.gpsimd.tensor_add(out=den2[:], in0=lvh[:], in1=lvv[:])
                  d2e = o_new()
                  nc.vector.tensor_scalar_add(d2e[:], den2[:], EPS)
                  rden2 = o_new()
                  nc.vector.reciprocal(rden2[:], d2e[:])
                  dconf = o_new()
                  nc.gpsimd.tensor_mul(out=dconf[:], in0=alvd[:], in1=rden2[:])
                  dma_out('dconf', dconf[:])

                  # ---- local mean / variance ----
                  lm_ps = convO('box3')
                  lmS = o_copy_out('lmean', lm_ps, eng='s')
                  lm2 = o_new()
                  nc.scalar.activation(lm2[:], lmS[:], AF.Square)
                  lq_ps = convO('box3', rhs=bsqr)
                  lvar = o_new()
                  nc.vector.scalar_tensor_tensor(out=h3(lvar[:]), in0=h3(lm2[:]),
                                                 scalar=-1.0, in1=ps3(lq_ps, NO),
                                                 op0=A.mult, op1=A.add)
                  dma_out('lvar', lvar[:])

    nc.compile()
    return nc


def _get_program(loop=1, timing=False):
    key = (loop, timing)
    if key not in _PROGRAM:
        _PROGRAM[key] = _build_program(loop, timing)
    return _PROGRAM[key]


def _host_constants():
    def kmajor(b):
        n, k, m = b.shape
        return np.ascontiguousarray(np.transpose(b, (1, 0, 2)).reshape(k, n * m))

    consts = {
        "bandsO": kmajor(BANDS_O),
        "bandsE": kmajor(BANDS_E),
        "bandsB": kmajor(BANDS_B),
    }
    # strip row k <-> image row (k - 3): parity (k+1)%2
    # strip col c <-> image col (c - 6): parity (c)%2
    rb = np.zeros((MO, 2 * W), np.float32)
    rb[:, 0:W] = _tile_pattern(_mask_pattern('r'), MO, W)
    rb[:, W:2 * W] = _tile_pattern(_mask_pattern('b'), MO, W)
    consts["rbmask"] = rb
    consts["gmask"] = _tile_pattern(_mask_pattern('g'), MO, W)
    m7 = np.zeros((7, MO, W), np.float32)
    for i, nm in enumerate(['r', 'g', 'b', 'gr', 'gb', 'row', 'col']):
        m7[i] = _tile_pattern(_mask_pattern(nm), MO, W)
    consts["masks7"] = m7
    return consts


def _in_maps(bayer):
    consts = _host_constants()

    def kmajor(bnd):
        n, k, mm = bnd.shape
        return np.ascontiguousarray(np.transpose(bnd, (1, 0, 2)).reshape(k, n * mm))

    padded = np.pad(bayer[:, 0], ((0, 0), (PH, PH), (PW, PW)), mode='reflect')
    in_maps = []
    for c in range(NCORES):
        b, j = divmod(c, CORES_PER_BATCH)
        strip = padded[b, j * RPC: j * RPC + SR, :]
        m = dict(consts)
        m["xs"] = np.ascontiguousarray(strip)
        if j == 0 or j == CORES_PER_BATCH - 1:
            bb = np.stack([_BB_STD,
                           _bb_variant('top') if j == 0 else _BB_STD,
                           _bb_variant('bot') if j == CORES_PER_BATCH - 1 else _BB_STD])
            m["bandsB"] = kmajor(bb)
        in_maps.append(m)
    return in_maps


def kernel(bayer: np.ndarray) -> np.ndarray:
    from concourse.bass_utils import run_bass_kernel_spmd

    bayer = np.asarray(bayer, np.float32)
    assert bayer.shape == (B, 1, H, W), bayer.shape
    nc = _get_program()
    res = run_bass_kernel_spmd(nc, _in_maps(bayer), list(range(NCORES)))
    out = np.zeros((B, 52, H, W), np.float32)
    for c in range(NCORES):
        b, j = divmod(c, CORES_PER_BATCH)
        out[b, :, j * RPC:(j + 1) * RPC, :] = res.results[c]["out"]
    return out

